# revision 1
# baseline (speedup 1.0000x reference)
"""Trainium2 Bass kernel for DemoGraphNet (2-layer GCN + mean-pool + MLP head).

Self-contained: hardcodes problem shapes and the 8-core sharding strategy.

v5 = v4 "late-weight" + balanced node placement + 16-granular bucket padding.

Late-weight: aggregation commutes with the layer weight matmul
(A_hat @ (X W) = (A_hat @ X) W), so each layer gathers RAW feature rows
(layer 1: xs = D^-1/2 x from a replicated bf16 input table -- no phase A,
no layer-1 AllGather; layer 2: h1s rows AllGathered once) and applies the
128x128 weight + bias + relu per 128-node dst window after aggregation,
in [dst, ch] orientation (which feeds the layer-2 table write and the
batch-one-hot pooling matmul directly).

Balanced placement: nodes are dealt to the 784 global (core, window) slots
in a degree-sorted snake, so per-(window, src-block) bucket counts are
near-uniform across cores and the SPMD max-quota padding collapses
(~35% -> ~10% padded gather descriptors).  Bucket quotas round to 16
(DMA packet granule) instead of 128; 128-edge chunks therefore cross
bucket boundaries at statically-known positions: a boundary chunk scatters
into two adjacent windows with two one-hot matmuls.  Window parity picks
the S one-hot band: dst code = slot + 128*(w%2), compared against a
0..255 iota band, so adjacent windows never alias inside a chunk.

Edges live on their dst core (self-loops appended).  dma_gather (SWDGE)
fetches 256B rows into [edge x chan] SBUF tiles.  Gathered rows carry
inv_sqrt[src]; window finish multiplies inv_sqrt[dst].  Pooled per-graph
sums are AllReduced; the tiny MLP head runs replicated on every core.
"""

import math
import os
import sys

sys.path.insert(0, "/opt/trn_rl_repo")

import numpy as np
import ml_dtypes

import concourse.bass as bass
import concourse.mybir as mybir
import concourse.tile as tile
import concourse.bacc as bacc
from concourse import library_config
from concourse.bass_utils import run_bass_kernel_spmd

BF16 = ml_dtypes.bfloat16
F32 = np.float32


class Cfg:
    def __init__(self, n_nodes, n_graphs, st_w, n_cores=8, hid=128, out_c=8):
        assert n_nodes % n_cores == 0
        self.N = n_nodes
        self.G = n_graphs
        self.C = n_cores
        self.HID = hid
        self.OUT = out_c
        self.NPC = n_nodes // n_cores          # nodes per core
        self.WIN = 128                          # dst window width
        self.NW = math.ceil(self.NPC / 128)     # windows per core
        self.PAD_NPC = self.NW * 128
        self.ST_W = st_w                        # windows per supertile
        self.NST = math.ceil(self.NW / st_w)
        self.TBL_N = n_cores * self.PAD_NPC     # padded table rows
        self.NBLK = max(1, math.ceil(self.TBL_N / 32768))
        self.BLKROWS = math.ceil(self.TBL_N / self.NBLK / 128) * 128


CFG = Cfg(n_nodes=100000, n_graphs=256, st_w=4)


# ----------------------------------------------------------------- host prep
def _host_prep(cfg, x, edge_index, batch):
    """Shard + marshal inputs. Index bookkeeping plus input re-encoding
    (node->slot placement, scaling x rows by inv_sqrt and casting to bf16);
    every matmul and all aggregation FLOPs run on device."""
    N, C = cfg.N, cfg.C
    NPC, WIN, NW, ST_W, NST = cfg.NPC, cfg.WIN, cfg.NW, cfg.ST_W, cfg.NST
    NBLK, BLKROWS, PAD_NPC = cfg.NBLK, cfg.BLKROWS, cfg.PAD_NPC
    TBL_N = cfg.TBL_N

    src = np.asarray(edge_index[0], dtype=np.int64)
    dst = np.asarray(edge_index[1], dtype=np.int64)
    batch = np.asarray(batch, dtype=np.int64)
    x = np.asarray(x, dtype=np.float32)

    deg = (np.bincount(dst, minlength=N) + 1).astype(np.float64)
    inv_sqrt = (1.0 / np.sqrt(deg)).astype(np.float32)

    # ---- balanced node -> slot placement.  Pass 1: degree-sorted snake
    # over all (core, window) slots equalizes window totals.  Pass 2:
    # within each src-block's quarter, re-deal nodes to windows matching
    # per-block in-degree vectors to the per-round target, so per-(window,
    # src-block) bucket counts are near-uniform across cores too.
    NWT = TBL_N // WIN                      # total windows (all cores)
    order = np.argsort(-deg, kind="stable")
    slot_of_node = np.empty(N, dtype=np.int64)
    nfull = N // NWT
    for r in range(nfull):
        nodes_r = order[r * NWT:(r + 1) * NWT]
        wins = np.arange(NWT) if r % 2 == 0 else np.arange(NWT)[::-1]
        slot_of_node[nodes_r] = wins * WIN + r
    rem = N - nfull * NWT
    if rem:
        nodes_r = order[nfull * NWT:]
        wins = np.arange(rem) if nfull % 2 == 0 else NWT - 1 - np.arange(rem)
        slot_of_node[nodes_r] = wins * WIN + nfull

    # per-node per-src-block in-degree under pass-1 source placement
    # (pass 2 keeps every node inside its quarter, so src blocks are stable)
    src_blk1 = slot_of_node[src] // BLKROWS
    dvec = np.zeros((N, NBLK), dtype=np.int64)
    np.add.at(dvec, (dst, src_blk1), 1)

    wpq = BLKROWS // WIN                    # windows per quarter
    for q in range(NBLK):
        qwins = np.arange(q * wpq, (q + 1) * wpq)
        qnodes = np.nonzero(slot_of_node // BLKROWS == q)[0]
        qn = qnodes[np.argsort(-deg[qnodes], kind="stable")]
        dv = dvec[qn].astype(np.float64)            # [nq, NBLK]
        nq = len(qn)
        S = np.zeros((wpq, NBLK))
        fill = np.zeros(wpq, dtype=np.int64)
        new_slot = np.empty(nq, dtype=np.int64)
        pos = 0
        rnd = 0
        while pos < nq:
            take = min(wpq, nq - pos)
            rn = np.arange(pos, pos + take)
            avail = np.ones(wpq, dtype=bool)
            avail[fill >= WIN] = False
            # deficit-matching: most-imbalanced nodes first, window with
            # the largest deficit in the node's dominant block
            node_order = rn[np.argsort(-np.abs(dv[rn] - dv[rn].mean(0)).max(1))]
            target = S[avail].mean(0) if avail.any() else S.mean(0)
            D = (target[None, :] - S)                # deficit per window
            for i in node_order:
                b = int(np.argmax(dv[i] - dv[rn].mean(0)))
                cand = np.nonzero(avail)[0]
                wsel = cand[np.argmax(D[cand, b])]
                new_slot[i] = qwins[wsel] * WIN + fill[wsel]
                S[wsel] += dv[i]
                D[wsel] -= dv[i]
                fill[wsel] += 1
                avail[wsel] = fill[wsel] < WIN
            pos += take
            rnd += 1
        slot_of_node[qn] = new_slot

    node_of_slot = np.full(TBL_N, -1, dtype=np.int64)
    node_of_slot[slot_of_node] = np.arange(N)
    valid_slot = node_of_slot >= 0

    # self-loops (the +I in A_hat) are NOT streamed as edges: each window
    # finish adds its own rows via an identity matmul instead (keeps the
    # per-(window, src-block) bucket counts balanced across cores).
    src_all = src
    dst_all = dst

    src_slot = slot_of_node[src_all]
    dst_slotg = slot_of_node[dst_all]
    core = dst_slotg // PAD_NPC
    tblrow = src_slot
    blk = tblrow // BLKROWS
    w_loc = (dst_slotg % PAD_NPC) // WIN
    slot_in_win = dst_slotg % WIN

    # bucket quotas: max count over cores, rounded to 16
    key = (core * NW + w_loc) * NBLK + blk
    counts = np.bincount(key, minlength=C * NW * NBLK).reshape(C, NW, NBLK)
    quota = counts.max(axis=0)
    quota = ((quota + 15) // 16) * 16          # [NW, NBLK]

    # segment layout: stream ordered (st, blk, w); segments pad to 128
    bucket_base = np.zeros((NW, NBLK), dtype=np.int64)
    seg_off = np.zeros((NST, NBLK), dtype=np.int64)
    seg_len = np.zeros((NST, NBLK), dtype=np.int64)
    st_meta = []                                # per st: emission metadata
    pos = 0
    for st in range(NST):
        ws = list(range(st * ST_W, min((st + 1) * ST_W, NW)))
        seg_groups = {}
        for b in range(NBLK):
            seg_off[st, b] = pos
            bounds = []
            p0 = pos
            for w in ws:
                bucket_base[w, b] = pos
                bounds.append((w, pos - p0, pos - p0 + quota[w, b]))
                pos += quota[w, b]
            L = (pos - p0 + 127) // 128 * 128
            pos = p0 + L
            seg_len[st, b] = L
            gci0 = p0 // 128
            # classify chunks: interior (one window) vs boundary (two)
            groups = []          # (mode, gci, gl, [(k_or_band, w)...])
            k = 0
            nch_seg = L // 128
            while k < nch_seg:
                lo, hi = 128 * k, 128 * (k + 1)
                ov = [w for (w, s, e) in bounds if s < hi and lo < e]
                assert 1 <= len(ov) <= 2, (st, b, k, ov)
                if len(ov) == 2:
                    weven = ov[0] if ov[0] % 2 == 0 else ov[1]
                    wodd = ov[1] if ov[1] % 2 == 1 else ov[0]
                    assert weven % 2 == 0 and wodd % 2 == 1
                    groups.append(("bnd", gci0 + k, 1,
                                   [(0, weven), (1, wodd)]))
                    k += 1
                else:
                    w = ov[0]
                    j = k
                    while j < nch_seg and j - k < 4:
                        lo2, hi2 = 128 * j, 128 * (j + 1)
                        ov2 = [ww for (ww, s, e) in bounds
                               if s < hi2 and lo2 < e]
                        if ov2 != [w]:
                            break
                        j += 1
                    groups.append(("int", gci0 + k, j - k,
                                   [(kk, w) for kk in range(j - k)]))
                    k = j
            seg_groups[b] = groups
        # first/last matmul flags per window within this st
        totals = {}
        for b in range(NBLK):
            for (_, _, _, mms) in seg_groups[b]:
                for (_, w) in mms:
                    totals[w] = totals.get(w, 0) + 1
        st_meta.append(dict(groups=seg_groups, totals=totals))
    EP = pos
    NCH = EP // 128

    cnt = np.bincount(batch, minlength=cfg.G).astype(np.float32)

    # replicated layer-1 gather table: xs = inv_sqrt * x in slot order, bf16
    xs = (x * inv_sqrt[:, None]).astype(BF16)
    xs_pad = np.zeros((TBL_N, cfg.HID), dtype=BF16)
    xs_pad[valid_slot] = xs[node_of_slot[valid_slot]]

    inv_slot = np.ones(TBL_N, dtype=np.float32)
    inv_slot[valid_slot] = inv_sqrt[node_of_slot[valid_slot]]
    batch_slot = np.full(TBL_N, -1.0, dtype=np.float32)
    batch_slot[valid_slot] = batch[node_of_slot[valid_slot]].astype(np.float32)

    in_maps = []
    okey = ((w_loc // ST_W) * NBLK + blk) * NW + w_loc
    for c in range(C):
        sel = np.nonzero(core == c)[0]
        e_okey = okey[sel]
        e_row = tblrow[sel]
        order_e = np.lexsort((e_row, e_okey))
        sel = sel[order_e]
        # rank within bucket
        bkey = (w_loc[sel] * NBLK + blk[sel])
        change = np.ones(len(sel), dtype=bool)
        change[1:] = bkey[1:] != bkey[:-1]
        gstart = np.maximum.accumulate(np.where(change, np.arange(len(sel)), 0))
        rank = np.arange(len(sel)) - gstart
        posn = bucket_base[w_loc[sel], blk[sel]] + rank

        e_idx16 = np.zeros(EP, dtype=np.int16)
        e_dst = np.full(EP, -1.0, dtype=np.float32)
        e_idx16[posn] = (tblrow[sel] - blk[sel] * BLKROWS).astype(np.int16)
        e_dst[posn] = (slot_in_win[sel]
                       + WIN * (w_loc[sel] % 2)).astype(np.float32)

        idx_img = np.tile(e_idx16.reshape(-1, 16).T, (8, 1)).copy()
        dst_col = e_dst.reshape(NCH, 128).T.astype(BF16)

        lo = c * PAD_NPC
        inv_own = inv_slot[lo:lo + PAD_NPC]
        inv_pp = inv_own.reshape(NW, 128).T.copy()
        inv_b = np.broadcast_to(inv_own, (128, PAD_NPC)).astype(BF16).copy()
        batch_pp = batch_slot[lo:lo + PAD_NPC].reshape(NW, 128).T.copy()

        in_maps.append({
            "xs": xs_pad, "xs_own": xs_pad[lo:lo + PAD_NPC].copy(),
            "idx": idx_img, "dstc": dst_col,
            "identb": np.eye(128, dtype=np.float32).astype(BF16),
            "invpp": inv_pp, "invb": inv_b, "batchpp": batch_pp,
            "iota256": np.broadcast_to(
                np.tile(np.arange(256, dtype=np.float32), 5),
                (128, 1280)).astype(BF16).copy(),
            "iotag": np.broadcast_to(
                np.arange(cfg.G, dtype=np.float32), (128, cfg.G)).astype(BF16).copy(),
            "cnt": cnt[None, :].copy(),
            "ones1": np.ones((1, 128), dtype=np.float32),
        })

    meta = dict(EP=EP, NCH=NCH, seg_off=seg_off, seg_len=seg_len,
                st_meta=st_meta)
    return in_maps, meta


def _add_weights(cfg, in_maps, W1, b1, W2, b2, Wh1, bh1, Wh2, bh2):
    wts = {
        "W1": np.asarray(W1, F32).astype(BF16),
        "b1": np.asarray(b1, F32).reshape(1, -1).astype(BF16),
        "W2": np.asarray(W2, F32).astype(BF16),
        "b2": np.asarray(b2, F32).reshape(1, -1).astype(BF16),
        "Wh1": np.asarray(Wh1, F32), "bh1": np.asarray(bh1, F32).reshape(-1, 1),
        "Wh2": np.asarray(Wh2, F32), "bh2": np.asarray(bh2, F32).reshape(-1, 1),
    }
    for m in in_maps:
        m.update(wts)


# ------------------------------------------------------------- program build
def _build(cfg, meta):
    NW, ST_W, NST, NBLK = cfg.NW, cfg.ST_W, cfg.NST, cfg.NBLK
    NCH, EP = meta["NCH"], meta["EP"]
    HID, G = cfg.HID, cfg.G
    bf = mybir.dt.bfloat16
    f32 = mybir.dt.float32

    no_coll = os.environ.get("GNN_NO_COLL") == "1"
    no_gather = os.environ.get("GNN_NO_GATHER") == "1"
    nc = bacc.Bacc("TRN2", target_bir_lowering=False, debug=False,
                   num_devices=cfg.C, num_swdge_queues=4)
    P = {}
    def param(name, shape, dt=f32):
        P[name] = nc.declare_dram_parameter(name, list(shape), dt, isOutput=False)
        return P[name]

    param("xs", [cfg.TBL_N, HID], bf)
    param("xs_own", [cfg.PAD_NPC, HID], bf)
    param("identb", [128, 128], bf)
    param("idx", [128, EP // 16], mybir.dt.int16)
    param("dstc", [128, NCH], bf)
    param("invpp", [128, NW]); param("invb", [128, cfg.PAD_NPC], bf)
    param("batchpp", [128, NW])
    param("iota256", [128, 1280], bf); param("iotag", [128, G], bf)
    param("cnt", [1, G]); param("ones1", [1, 128])
    param("W1", [HID, HID], bf); param("b1", [1, HID], bf)
    param("W2", [HID, HID], bf); param("b2", [1, HID], bf)
    param("Wh1", [HID, HID]); param("bh1", [HID, 1])
    param("Wh2", [HID, cfg.OUT]); param("bh2", [cfg.OUT, 1])
    t_out = nc.declare_dram_parameter("out", [cfg.OUT, G], f32, isOutput=True)

    replica = [list(range(cfg.C))]

    with tile.TileContext(nc) as tc:
        with (
            tc.tile_pool(name="const", bufs=1) as cp,
            tc.tile_pool(name="mtiles", bufs=8) as mp,
            tc.tile_pool(name="stiles", bufs=12) as sp,
            tc.tile_pool(name="evac", bufs=6) as ep,
            tc.tile_pool(name="psw", bufs=4, space="PSUM") as psw,
            tc.tile_pool(name="psa", bufs=3, space="PSUM") as psa,
            tc.tile_pool(name="psg", bufs=1, space="PSUM") as psg,
            tc.tile_pool(name="dram", bufs=1, space="DRAM") as dp,
        ):
            nc.gpsimd.load_library(library_config.mlp)

            # ---- constants / metadata to SBUF (idx first: gathers need it)
            def load(name, shape, dt=f32):
                t = cp.tile(list(shape), dt, tag=f"c_{name}", name=f"c_{name}")
                nc.sync.dma_start(t[:], P[name][:])
                return t
            idx_sb = load("idx", [128, EP // 16], mybir.dt.int16)
            ident_sb = load("identb", [128, 128], bf)
            dst_sb = load("dstc", [128, NCH], bf)
            iota_sb = load("iota256", [128, 1280], bf)
            invb_sb = load("invb", [128, cfg.PAD_NPC], bf)
            invs_sb = load("invpp", [128, NW])
            batch_sb = load("batchpp", [128, NW])
            iotag_sb = load("iotag", [128, G], bf)
            cnt_sb = load("cnt", [1, G])
            ones1_sb = load("ones1", [1, 128])
            W1_sb = load("W1", [HID, HID], bf); b1_sb = load("b1", [1, HID], bf)
            W2_sb = load("W2", [HID, HID], bf); b2_sb = load("b2", [1, HID], bf)
            Wh1_sb = load("Wh1", [HID, HID]); bh1_sb = load("bh1", [HID, 1])
            Wh2_sb = load("Wh2", [HID, cfg.OUT]); bh2_sb = load("bh2", [cfg.OUT, 1])
            ones1b_sb = cp.tile([1, 128], bf, tag="c_ones1b")
            nc.vector.tensor_copy(out=ones1b_sb[:], in_=ones1_sb[:])

            # layer-2 gather table (h1s rows), own shard + AllGathered full.
            # Raw dram tensors, manually registered in the DGE table (SWDGE
            # descriptor relocation needs it; device crash otherwise).
            tbl2_own = nc.dram_tensor("tbl2own", [cfg.PAD_NPC, HID], bf)
            tbl2 = nc.dram_tensor("tbl2", [cfg.TBL_N, HID], bf)
            for t in (P["xs"], tbl2):
                mloc = nc.lookup_mloc(t)
                if mloc.table_entry_id is None:
                    mloc.table_entry_id = len(nc.dge_table) + 1
                    nc.dge_table.append(mloc.name)

            ps_pool = psg.tile([128, G], f32, space="PSUM", tag="g")

            # self-loop rows for every own window, loaded in ONE dma per
            # layer: [128 node-in-window, NW, 128 ch]
            srow_sb = [cp.tile([128, NW, 128], bf, tag=f"c_srow{l}",
                               name=f"c_srow{l}") for l in range(2)]
            nc.sync.dma_start(
                srow_sb[0][:],
                P["xs_own"][:].rearrange("(w p) c -> p w c", p=128))

            # ---- aggregation sweep (shared for both layers)
            def agg_layer(layer):
                tbl = P["xs"] if layer == 0 else tbl2
                for st in range(NST):
                    sm = meta["st_meta"][st]
                    remaining = dict(sm["totals"])
                    full = sm["totals"]
                    wt = {w: psw.tile([128, 128], f32, space="PSUM",
                                      tag="win", name=f"wt_{layer}_{w}")
                          for w in range(st * ST_W, min((st + 1) * ST_W, NW))}
                    def wslot(w):
                        return wt[w][:]
                    m_tiles = {}
                    for b in range(NBLK):
                        off = int(meta["seg_off"][st, b])
                        ln = int(meta["seg_len"][st, b])
                        if ln == 0:
                            continue
                        mt = mp.tile([128, ln // 128, 128], bf, tag="mtile",
                                     name=f"m_{layer}_{st}_{b}")
                        m_tiles[b] = (mt, off // 128)
                        if no_gather:
                            nc.gpsimd.memset(mt[:], 0.5)
                        else:
                            nc.gpsimd.dma_gather(
                                mt[:],
                                tbl[b * cfg.BLKROWS:
                                    min((b + 1) * cfg.BLKROWS, cfg.TBL_N), :],
                                idx_sb[:, off // 16: off // 16 + ln // 16],
                                ln, ln, HID,
                                single_packet=False, queue_num=b % 4)
                    for b in range(NBLK):
                        if b not in m_tiles:
                            continue
                        mt, mbase = m_tiles[b]
                        for (mode, gci0, gl, mms) in sm["groups"][b]:
                            sq = sp.tile([128, max(gl, 2) if mode == "bnd"
                                          else gl, 128], bf, tag="s",
                                         name=f"s_{layer}_{st}_{gci0}_{mode}")
                            if mode == "int":
                                w = mms[0][1]
                                p = w % 2
                                nc.vector.tensor_tensor(
                                    out=sq[:],
                                    in0=dst_sb[:, gci0:gci0 + gl].to_broadcast(
                                        [128, gl, 128]),
                                    in1=iota_sb[:, p * 128:
                                                p * 128 + gl * 256].rearrange(
                                        "p (c j) -> p c j", j=256)[:, :, 0:128],
                                    op=mybir.AluOpType.is_equal)
                            else:
                                nc.vector.tensor_tensor(
                                    out=sq[:],
                                    in0=dst_sb[:, gci0:gci0 + 1].to_broadcast(
                                        [128, 2, 128]),
                                    in1=iota_sb[:, 0:256].rearrange(
                                        "p (c j) -> p c j", j=128),
                                    op=mybir.AluOpType.is_equal)
                            for (k, w) in mms:
                                first = remaining[w] == full[w]
                                last = remaining[w] == 1
                                remaining[w] -= 1
                                gci = gci0 + (k if mode == "int" else 0)
                                nc.tensor.matmul(out=wslot(w),
                                                 lhsT=mt[:, gci - mbase, :],
                                                 rhs=sq[:, k, :],
                                                 start=first, stop=False,
                                                 skip_group_check=True)
                                if last:
                                    finish_window(layer, w, wslot(w))

            def finish_window(layer, w, ps_w):
                # ps_w: [ch_in, dst] aggregate of gathered rows (carrying
                # inv_sqrt[src]); add the self-loop rows (own window, via
                # identity matmul transpose-accumulate), then evac applies
                # inv_sqrt[dst]; the layer weight + bias + relu run per
                # window in [dst, ch] layout.
                w0 = w * 128
                nc.tensor.matmul(out=ps_w, lhsT=srow_sb[layer][:, w, :],
                                 rhs=ident_sb[:],
                                 start=False, stop=True,
                                 skip_group_check=True)
                aggs = ep.tile([128, 128], bf, tag="aggs")
                nc.vector.tensor_tensor(out=aggs[:], in0=ps_w[:],
                                        in1=invb_sb[:, w0:w0 + 128],
                                        op=mybir.AluOpType.mult)
                W_sb = W1_sb if layer == 0 else W2_sb
                bias = b1_sb if layer == 0 else b2_sb
                psz = psa.tile([128, HID], f32, space="PSUM", tag="a")
                nc.tensor.matmul(out=psz[:], lhsT=aggs[:], rhs=W_sb[:],
                                 start=True, stop=False)
                nc.tensor.matmul(out=psz[:], lhsT=ones1b_sb[:],
                                 rhs=bias[:], start=False, stop=True)
                if layer == 0:
                    # h1s = inv_sqrt[dst]*relu(z) = relu(inv_sqrt*z), node-major
                    t2 = ep.tile([128, HID], bf, tag="t2")
                    nc.scalar.activation(t2[:], psz[:],
                                         mybir.ActivationFunctionType.Relu,
                                         scale=invs_sb[:, w:w + 1])
                    nc.sync.dma_start(tbl2_own[w0:w0 + 128, :], t2[:])
                else:
                    # h2 window [node, ch] -> pooled one-hot accumulate
                    h2w = ep.tile([128, HID], bf, tag="h2w")
                    nc.scalar.activation(h2w[:], psz[:],
                                         mybir.ActivationFunctionType.Relu)
                    sg = sp.tile([128, G], bf, tag="sg")
                    nc.vector.tensor_tensor(
                        out=sg[:],
                        in0=batch_sb[:, w:w + 1].to_broadcast([128, G]),
                        in1=iotag_sb[:],
                        op=mybir.AluOpType.is_equal)
                    nc.tensor.matmul(out=ps_pool[:], lhsT=h2w[:], rhs=sg[:],
                                     start=(w == 0), stop=(w == NW - 1),
                                     skip_group_check=True)

            agg_layer(0)
            nc.sync.dma_start(
                srow_sb[1][:],
                tbl2_own[:].rearrange("(w p) c -> p w c", p=128))
            if no_coll:
                nc.sync.dma_start(tbl2[:cfg.PAD_NPC, :], tbl2_own[:])
            else:
                nc.gpsimd.collective_compute(
                    "AllGather", mybir.AluOpType.bypass,
                    ins=[tbl2_own[:]], outs=[tbl2[:]],
                    replica_groups=replica)
            agg_layer(1)

            # ---- pooled mean + head (replicated on every core)
            pooled_l = ep.tile([128, G], f32, tag="pool")
            nc.vector.tensor_copy(out=pooled_l[:], in_=ps_pool[:])
            ar_in = dp.tile([128, G], f32, tag="arin")
            ar_out = dp.tile([128, G], f32, addr_space="Shared", tag="arout")
            nc.sync.dma_start(ar_in[:], pooled_l[:])
            if no_coll:
                nc.sync.dma_start(ar_out[:], ar_in[:])
            else:
                nc.gpsimd.collective_compute(
                    "AllReduce", mybir.AluOpType.add,
                    ins=[ar_in.opt()], outs=[ar_out.opt()],
                    replica_groups=replica)
            pooled = ep.tile([128, G], f32, tag="pool")
            nc.sync.dma_start(pooled[:], ar_out[:])

            psc = psg.tile([128, G], f32, space="PSUM", tag="g")
            nc.tensor.matmul(out=psc[:], lhsT=ones1_sb[:], rhs=cnt_sb[:],
                             start=True, stop=True)
            cntb = ep.tile([128, G], f32, tag="pool")
            nc.vector.tensor_scalar_max(out=cntb[:], in0=psc[:], scalar1=1.0)
            invc = ep.tile([128, G], f32, tag="pool")
            nc.vector.reciprocal(invc[:], cntb[:])
            pmean = ep.tile([128, G], f32, tag="pool")
            nc.vector.tensor_tensor(out=pmean[:], in0=pooled[:], in1=invc[:],
                                    op=mybir.AluOpType.mult)

            psh1 = psg.tile([128, G], f32, space="PSUM", tag="g")
            nc.tensor.matmul(out=psh1[:], lhsT=Wh1_sb[:], rhs=pmean[:],
                             start=True, stop=True)
            relu1 = ep.tile([128, G], f32, tag="pool")
            nc.scalar.activation(relu1[:], psh1[:],
                                 mybir.ActivationFunctionType.Relu,
                                 bias=bh1_sb[:, 0:1])
            psh2 = psg.tile([cfg.OUT, G], f32, space="PSUM", tag="g")
            nc.tensor.matmul(out=psh2[:], lhsT=Wh2_sb[:], rhs=relu1[:],
                             start=True, stop=True)
            out_sb = ep.tile([cfg.OUT, G], f32, tag="out")
            nc.vector.tensor_scalar_add(out=out_sb[:], in0=psh2[:],
                                        scalar1=bh2_sb[:, 0:1])
            nc.sync.dma_start(t_out[:], out_sb[:])

    nc.compile()
    return nc


# ----------------------------------------------------------------- entry
def _run(inputs, cfg=CFG, trace=False):
    in_maps, meta = _host_prep(cfg, inputs["x"], inputs["edge_index"],
                               inputs["batch"])
    _add_weights(cfg, in_maps,
                 inputs["W1"], inputs["b1"], inputs["W2"], inputs["b2"],
                 inputs["Wh1"], inputs["bh1"], inputs["Wh2"], inputs["bh2"])
    nc = _build(cfg, meta)
    res = run_bass_kernel_spmd(nc, in_maps, list(range(cfg.C)), trace=trace)
    out = np.ascontiguousarray(np.asarray(res.results[0]["out"]).T)
    return out, res


def kernel(**inputs) -> np.ndarray:
    out, _ = _run(inputs, CFG, trace=False)
    return out



# revision 19
# speedup vs baseline: 1.5320x; 1.5320x over previous
"""Trainium2 Bass kernel for DemoGraphNet (2-layer GCN + mean-pool + MLP head).

Self-contained: hardcodes problem shapes and the 8-core sharding strategy.

v7 = v5 aggregation pipeline + quarter-chunked overlapped AllGather +
single packed constant blob (2 input args total).

Late-weight: aggregation commutes with the layer weight matmul
(A_hat @ (X W) = (A_hat @ X) W), so each layer gathers RAW feature rows
(layer 1: xs = D^-1/2 x from a replicated bf16 input table; layer 2: h1s
rows AllGathered per quarter) and applies the 128x128 weight + bias +
relu per 128-node dst window after aggregation in [dst, ch] orientation.

Quarter-chunked AllGather: the gather-table row space is QUARTER-major
(4 supertile-aligned window groups, each <= 32768 rows so int16 gather
indices still reach them).  Each quarter AllGathers as soon as layer 1
finishes its windows, overlapping the collective with the layer-1 tail;
layer-2 gather blocks == quarters, so only the last quarter's collective
can stall the layer-2 pipeline (~1/4 of the old exposure).

Packed blob: per-exec dispatch cost on this runtime scales with the
argument count (~70us/arg), dwarfing most device-side effects, so every
constant (gather indices, dst codes, one-hot iota bands, inv-sqrt
tables, weights, the layer-1 self-row image) is packed host-side into
one [128, B] uint8 image, DMA'd once, and sliced/bitcast on SBUF.

Edges live on their dst core (self-loops via identity matmul).
dma_gather (SWDGE) fetches 256B rows (bucket-sorted ascending for HBM
row locality) into [edge x chan] SBUF tiles.  Gathered rows carry
inv_sqrt[src]; window finish multiplies inv_sqrt[dst].  Pooled
per-graph sums are AllReduced; the tiny MLP head runs replicated.
"""

import math
import os
import sys

sys.path.insert(0, "/opt/trn_rl_repo")

import numpy as np
import ml_dtypes

import concourse.bass as bass
import concourse.mybir as mybir
import concourse.tile as tile
import concourse.bacc as bacc
from concourse import library_config
from concourse.bass_utils import run_bass_kernel_spmd

BF16 = ml_dtypes.bfloat16
F32 = np.float32


class Cfg:
    def __init__(self, n_nodes, n_graphs, st_w, n_cores=8, hid=128, out_c=8):
        assert n_nodes % n_cores == 0
        self.N = n_nodes
        self.G = n_graphs
        self.C = n_cores
        self.HID = hid
        self.OUT = out_c
        self.NPC = n_nodes // n_cores          # nodes per core
        self.WIN = 128                          # dst window width
        self.NW = math.ceil(self.NPC / 128)     # windows per core
        self.PAD_NPC = self.NW * 128
        self.ST_W = st_w                        # windows per supertile
        self.NST = math.ceil(self.NW / st_w)
        self.TBL_N = n_cores * self.PAD_NPC     # padded table rows
        self.NBLK = 4
        # split the NST supertiles into 4 quarters (last the smallest);
        # table layout is quarter-major so each quarter AllGathers
        # independently and is a contiguous int16-indexable gather block
        base = self.NST // 4
        rem = self.NST - 4 * base
        self.QSTS = [base + (1 if i < rem else 0) for i in range(4)]
        self.QW0 = [0]
        for i in range(4):
            self.QW0.append(min(self.QW0[-1] + self.QSTS[i] * st_w, self.NW))
        self.QWIN = [self.QW0[i + 1] - self.QW0[i] for i in range(4)]
        self.QROWS = [q * 128 * n_cores for q in self.QWIN]
        self.QROW0 = [0]
        for i in range(4):
            self.QROW0.append(self.QROW0[-1] + self.QROWS[i])
        assert all(r <= 32768 for r in self.QROWS), self.QROWS
        self.WQ_OF_W = []
        for i in range(4):
            self.WQ_OF_W += [i] * self.QWIN[i]


CFG = Cfg(n_nodes=100000, n_graphs=256, st_w=4)


# ------------------------------------------------------- blob layout
def _blob_layout(cfg, EP, NCH):
    """Column-byte layout of the packed constant image [128, total]."""
    sec = [
        ("idx",     np.int16, EP // 16),
        ("identb",  BF16,     128),
        ("dstcb",   BF16,     NCH),
        ("invb",    BF16,     cfg.PAD_NPC),
        ("invpp",   F32,      cfg.NW),
        ("batchpp", F32,      cfg.NW),
        ("iota256", BF16,     1280),
        ("iotag",   BF16,     cfg.G),
        ("cnt",     F32,      cfg.G),
        ("ones1",   F32,      128),
        ("ones1b",  BF16,     128),
        ("W1",      BF16,     cfg.HID),
        ("b1",      BF16,     cfg.HID),
        ("W2",      BF16,     cfg.HID),
        ("b2",      BF16,     cfg.HID),
        ("Wh1",     F32,      cfg.HID),
        ("bh1",     F32,      1),
        ("Wh2",     F32,      cfg.OUT),
        ("bh2",     F32,      1),
        ("xsown",   BF16,     cfg.NW * cfg.HID),
    ]
    layout = {}
    off = 0
    for name, dt, cols in sec:
        nbytes = cols * np.dtype(dt).itemsize
        layout[name] = (off, dt, cols)
        off += (nbytes + 3) // 4 * 4
    return layout, off


def _pack_blob(cfg, EP, NCH, pieces):
    layout, total = _blob_layout(cfg, EP, NCH)
    blob = np.zeros((128, total), dtype=np.uint8)
    for name, (off, dt, cols) in layout.items():
        arr = np.ascontiguousarray(pieces[name].astype(dt))
        assert arr.ndim == 2 and arr.shape[1] == cols, (name, arr.shape, cols)
        p = arr.shape[0]
        blob[0:p, off:off + cols * np.dtype(dt).itemsize] = arr.view(np.uint8)
    return blob


# ----------------------------------------------------------------- host prep
def _host_prep(cfg, x, edge_index, batch):
    """Shard + marshal inputs. Index bookkeeping plus input re-encoding
    (node->slot placement, scaling x rows by inv_sqrt and casting to bf16);
    every matmul and all aggregation FLOPs run on device."""
    N, C = cfg.N, cfg.C
    NPC, WIN, NW, ST_W, NST = cfg.NPC, cfg.WIN, cfg.NW, cfg.ST_W, cfg.NST
    NBLK, PAD_NPC = cfg.NBLK, cfg.PAD_NPC
    TBL_N = cfg.TBL_N

    src = np.asarray(edge_index[0], dtype=np.int64)
    dst = np.asarray(edge_index[1], dtype=np.int64)
    batch = np.asarray(batch, dtype=np.int64)
    x = np.asarray(x, dtype=np.float32)

    deg = (np.bincount(dst, minlength=N) + 1).astype(np.float64)
    inv_sqrt = (1.0 / np.sqrt(deg)).astype(np.float32)

    # ---- balanced node -> slot placement.  Pass 1: degree-sorted snake
    # over all (core, window) slots equalizes window totals.  Pass 2:
    # within each src-block's quarter, re-deal nodes to windows matching
    # per-block in-degree vectors to the per-round target, so per-(window,
    # src-block) bucket counts are near-uniform across cores too.
    NWT = TBL_N // WIN                      # total windows (all cores)
    order = np.argsort(-deg, kind="stable")
    slot_of_node = np.empty(N, dtype=np.int64)
    nfull = N // NWT
    for r in range(nfull):
        nodes_r = order[r * NWT:(r + 1) * NWT]
        wins = np.arange(NWT) if r % 2 == 0 else np.arange(NWT)[::-1]
        slot_of_node[nodes_r] = wins * WIN + r
    rem = N - nfull * NWT
    if rem:
        nodes_r = order[nfull * NWT:]
        wins = np.arange(rem) if nfull % 2 == 0 else NWT - 1 - np.arange(rem)
        slot_of_node[nodes_r] = wins * WIN + nfull

    # per-node per-src-block in-degree under pass-1 source placement
    # (pass 2 keeps every node inside its quarter, so src blocks are stable)
    wq_of_w = np.asarray(cfg.WQ_OF_W, dtype=np.int64)   # local window -> blk
    src_blk1 = wq_of_w[(slot_of_node[src] // WIN) % NW]
    dvec = np.zeros((N, NBLK), dtype=np.int64)
    np.add.at(dvec, (dst, src_blk1), 1)

    gwin_all = np.arange(TBL_N // WIN)
    for q in range(NBLK):
        qwins = gwin_all[wq_of_w[gwin_all % NW] == q]
        wpq = len(qwins)
        qnodes = np.nonzero(
            wq_of_w[(slot_of_node // WIN) % NW] == q)[0]
        qn = qnodes[np.argsort(-deg[qnodes], kind="stable")]
        dv = dvec[qn].astype(np.float64)            # [nq, NBLK]
        nq = len(qn)
        S = np.zeros((wpq, NBLK))
        fill = np.zeros(wpq, dtype=np.int64)
        new_slot = np.empty(nq, dtype=np.int64)
        pos = 0
        rnd = 0
        while pos < nq:
            take = min(wpq, nq - pos)
            rn = np.arange(pos, pos + take)
            avail = np.ones(wpq, dtype=bool)
            avail[fill >= WIN] = False
            # deficit-matching: most-imbalanced nodes first, window with
            # the largest deficit in the node's dominant block
            node_order = rn[np.argsort(-np.abs(dv[rn] - dv[rn].mean(0)).max(1))]
            target = S[avail].mean(0) if avail.any() else S.mean(0)
            D = (target[None, :] - S)                # deficit per window
            for i in node_order:
                b = int(np.argmax(dv[i] - dv[rn].mean(0)))
                cand = np.nonzero(avail)[0]
                wsel = cand[np.argmax(D[cand, b])]
                new_slot[i] = qwins[wsel] * WIN + fill[wsel]
                S[wsel] += dv[i]
                D[wsel] -= dv[i]
                fill[wsel] += 1
                avail[wsel] = fill[wsel] < WIN
            pos += take
            rnd += 1
        slot_of_node[qn] = new_slot

    node_of_slot = np.full(TBL_N, -1, dtype=np.int64)
    node_of_slot[slot_of_node] = np.arange(N)
    valid_slot = node_of_slot >= 0

    # quarter-major table row for every (old-numbering) global slot
    g_of_s = np.arange(TBL_N) // WIN
    i_of_s = np.arange(TBL_N) % WIN
    c_of_s = g_of_s // NW
    w_of_s = g_of_s % NW
    q_of_s = wq_of_w[w_of_s]
    qw0 = np.asarray(cfg.QW0[:4], dtype=np.int64)
    qwin = np.asarray(cfg.QWIN, dtype=np.int64)
    qrow0 = np.asarray(cfg.QROW0[:4], dtype=np.int64)
    tablerow_of_slot = (qrow0[q_of_s] + c_of_s * qwin[q_of_s] * WIN
                        + (w_of_s - qw0[q_of_s]) * WIN + i_of_s)

    # self-loops (the +I in A_hat) are NOT streamed as edges: each window
    # finish adds its own rows via an identity matmul instead (keeps the
    # per-(window, src-block) bucket counts balanced across cores).
    src_slot = slot_of_node[src]
    dst_slotg = slot_of_node[dst]
    core = dst_slotg // PAD_NPC
    tblrow = tablerow_of_slot[src_slot]
    blk = wq_of_w[(src_slot // WIN) % NW]
    w_loc = (dst_slotg % PAD_NPC) // WIN
    slot_in_win = dst_slotg % WIN

    # bucket quotas: max count over cores, rounded to 16
    key = (core * NW + w_loc) * NBLK + blk
    counts = np.bincount(key, minlength=C * NW * NBLK).reshape(C, NW, NBLK)
    quota = counts.max(axis=0)
    quota = ((quota + 15) // 16) * 16          # [NW, NBLK]

    # segment layout: stream ordered (st, blk, w); segments pad to 128
    bucket_base = np.zeros((NW, NBLK), dtype=np.int64)
    seg_off = np.zeros((NST, NBLK), dtype=np.int64)
    seg_len = np.zeros((NST, NBLK), dtype=np.int64)
    st_meta = []                                # per st: emission metadata
    pos = 0
    for st in range(NST):
        ws = list(range(st * ST_W, min((st + 1) * ST_W, NW)))
        seg_groups = {}
        for b in range(NBLK):
            seg_off[st, b] = pos
            bounds = []
            p0 = pos
            for w in ws:
                bucket_base[w, b] = pos
                bounds.append((w, pos - p0, pos - p0 + quota[w, b]))
                pos += quota[w, b]
            L = (pos - p0 + 127) // 128 * 128
            pos = p0 + L
            seg_len[st, b] = L
            gci0 = p0 // 128
            # classify chunks: interior (one window) vs boundary (two)
            groups = []          # (mode, gci, gl, [(k_or_band, w)...])
            k = 0
            nch_seg = L // 128
            while k < nch_seg:
                lo, hi = 128 * k, 128 * (k + 1)
                ov = [w for (w, s, e) in bounds if s < hi and lo < e]
                assert 1 <= len(ov) <= 2, (st, b, k, ov)
                if len(ov) == 2:
                    weven = ov[0] if ov[0] % 2 == 0 else ov[1]
                    wodd = ov[1] if ov[1] % 2 == 1 else ov[0]
                    assert weven % 2 == 0 and wodd % 2 == 1
                    groups.append(("bnd", gci0 + k, 1,
                                   [(0, weven), (1, wodd)]))
                    k += 1
                else:
                    w = ov[0]
                    j = k
                    while j < nch_seg and j - k < 4:
                        lo2, hi2 = 128 * j, 128 * (j + 1)
                        ov2 = [ww for (ww, s, e) in bounds
                               if s < hi2 and lo2 < e]
                        if ov2 != [w]:
                            break
                        j += 1
                    groups.append(("int", gci0 + k, j - k,
                                   [(kk, w) for kk in range(j - k)]))
                    k = j
            seg_groups[b] = groups
        # first/last matmul flags per window within this st
        totals = {}
        for b in range(NBLK):
            for (_, _, _, mms) in seg_groups[b]:
                for (_, w) in mms:
                    totals[w] = totals.get(w, 0) + 1
        st_meta.append(dict(groups=seg_groups, totals=totals))
    EP = pos
    NCH = EP // 128

    cnt = np.bincount(batch, minlength=cfg.G).astype(np.float32)

    # replicated layer-1 gather table: xs = inv_sqrt * x, quarter-major rows
    xs = (x * inv_sqrt[:, None]).astype(BF16)
    xs_slot = np.zeros((TBL_N, cfg.HID), dtype=BF16)
    xs_slot[valid_slot] = xs[node_of_slot[valid_slot]]
    xs_pad = np.zeros((TBL_N, cfg.HID), dtype=BF16)
    xs_pad[tablerow_of_slot] = xs_slot

    inv_slot = np.ones(TBL_N, dtype=np.float32)
    inv_slot[valid_slot] = inv_sqrt[node_of_slot[valid_slot]]
    batch_slot = np.full(TBL_N, -1.0, dtype=np.float32)
    batch_slot[valid_slot] = batch[node_of_slot[valid_slot]].astype(np.float32)

    in_maps = []
    okey = ((w_loc // ST_W) * NBLK + blk) * NW + w_loc
    for c in range(C):
        sel = np.nonzero(core == c)[0]
        e_okey = okey[sel]
        e_row = tblrow[sel]
        order_e = np.lexsort((e_row, e_okey))
        sel = sel[order_e]
        # rank within bucket
        bkey = (w_loc[sel] * NBLK + blk[sel])
        change = np.ones(len(sel), dtype=bool)
        change[1:] = bkey[1:] != bkey[:-1]
        gstart = np.maximum.accumulate(np.where(change, np.arange(len(sel)), 0))
        rank = np.arange(len(sel)) - gstart
        posn = bucket_base[w_loc[sel], blk[sel]] + rank

        e_idx16 = np.zeros(EP, dtype=np.int16)
        e_dst = np.full(EP, -1.0, dtype=np.float32)
        e_idx16[posn] = (tblrow[sel] - qrow0[blk[sel]]).astype(np.int16)
        e_dst[posn] = (slot_in_win[sel]
                       + WIN * (w_loc[sel] % 2)).astype(np.float32)

        idx_img = np.tile(e_idx16.reshape(-1, 16).T, (8, 1)).copy()
        dst_colb = np.ascontiguousarray(e_dst.reshape(NCH, 128).T).astype(BF16)

        lo = c * PAD_NPC
        inv_own = inv_slot[lo:lo + PAD_NPC]
        inv_pp = inv_own.reshape(NW, 128).T.copy()
        inv_b = np.broadcast_to(inv_own, (128, PAD_NPC)).astype(BF16).copy()
        batch_pp = batch_slot[lo:lo + PAD_NPC].reshape(NW, 128).T.copy()
        # layer-1 self rows, pre-arranged [128 slot-in-window, NW*HID]
        xsown_img = (xs_slot[lo:lo + PAD_NPC]
                     .reshape(NW, 128, cfg.HID).transpose(1, 0, 2)
                     .reshape(128, NW * cfg.HID).copy())

        in_maps.append({
            "xs": xs_pad,
            "_pieces": {
                "idx": idx_img, "identb": np.eye(128, dtype=np.float32),
                "dstcb": dst_colb, "invb": inv_b, "invpp": inv_pp,
                "batchpp": batch_pp,
                "iota256": np.broadcast_to(
                    np.tile(np.arange(256, dtype=np.float32), 5),
                    (128, 1280)),
                "iotag": np.broadcast_to(
                    np.arange(cfg.G, dtype=np.float32), (128, cfg.G)),
                "cnt": cnt[None, :],
                "ones1": np.ones((1, 128), dtype=np.float32),
                "ones1b": np.ones((1, 128), dtype=np.float32),
                "xsown": xsown_img,
            },
        })

    meta = dict(EP=EP, NCH=NCH, seg_off=seg_off, seg_len=seg_len,
                st_meta=st_meta)
    return in_maps, meta


def _add_weights(cfg, in_maps, W1, b1, W2, b2, Wh1, bh1, Wh2, bh2):
    wts = {
        "W1": np.asarray(W1, F32), "b1": np.asarray(b1, F32).reshape(1, -1),
        "W2": np.asarray(W2, F32), "b2": np.asarray(b2, F32).reshape(1, -1),
        "Wh1": np.asarray(Wh1, F32),
        "bh1": np.asarray(bh1, F32).reshape(-1, 1),
        "Wh2": np.asarray(Wh2, F32),
        "bh2": np.asarray(bh2, F32).reshape(-1, 1),
    }
    for m in in_maps:
        pieces = m.pop("_pieces")
        pieces.update(wts)
        NCH = pieces["dstcb"].shape[1]
        EP = NCH * 128
        m["blob"] = _pack_blob(cfg, EP, NCH, pieces)


# ------------------------------------------------------------- program build
def _build(cfg, meta):
    NW, ST_W, NST, NBLK = cfg.NW, cfg.ST_W, cfg.NST, cfg.NBLK
    NCH, EP = meta["NCH"], meta["EP"]
    HID, G = cfg.HID, cfg.G
    bf = mybir.dt.bfloat16
    f32 = mybir.dt.float32
    u8 = mybir.dt.uint8

    no_coll = os.environ.get("GNN_NO_COLL") == "1"
    no_gather = os.environ.get("GNN_NO_GATHER") == "1"
    nc = bacc.Bacc("TRN2", target_bir_lowering=False, debug=False,
                   num_devices=cfg.C, num_swdge_queues=4)

    layout, blob_bytes = _blob_layout(cfg, EP, NCH)
    P_xs = nc.declare_dram_parameter("xs", [cfg.TBL_N, HID], bf,
                                     isOutput=False)
    P_blob = nc.declare_dram_parameter("blob", [128, blob_bytes], u8,
                                       isOutput=False)
    t_out = nc.declare_dram_parameter("out", [cfg.OUT, G], f32, isOutput=True)

    replica = [list(range(cfg.C))]
    mdt = {np.int16: mybir.dt.int16, BF16: bf, F32: f32}

    with tile.TileContext(nc) as tc:
        with (
            tc.tile_pool(name="const", bufs=1) as cp,
            tc.tile_pool(name="mtiles",
                         bufs=int(os.environ.get("GNN_MPBUFS", "8"))) as mp,
            tc.tile_pool(name="stiles", bufs=12) as sp,
            tc.tile_pool(name="evac", bufs=6) as ep,
            tc.tile_pool(name="psw",
                         bufs=int(os.environ.get("GNN_PSWBUFS", "4")),
                         space="PSUM") as psw,
            tc.tile_pool(name="psa", bufs=3, space="PSUM") as psa,
            tc.tile_pool(name="psg", bufs=1, space="PSUM") as psg,
            tc.tile_pool(name="dram", bufs=1, space="DRAM") as dp,
        ):
            nc.gpsimd.load_library(library_config.mlp)

            # ---- the packed constants, one DMA
            blob_sb = cp.tile([128, blob_bytes], u8, tag="c_blob",
                              name="c_blob")
            nc.sync.dma_start(blob_sb[:], P_blob[:])

            def view(name):
                off, dt, cols = layout[name]
                nb = cols * np.dtype(dt).itemsize
                return blob_sb[:, off:off + nb].bitcast(mdt[dt])

            idx_sb = view("idx")
            ident_sb = view("identb")
            dstb_sb = view("dstcb")
            invb_sb = view("invb")
            invs_sb = view("invpp")
            batch_sb = view("batchpp")
            iota256_sb = view("iota256")
            iotag_sb = view("iotag")
            cnt_v = view("cnt")
            ones1_v = view("ones1")
            ones1b_v = view("ones1b")
            W1_sb = view("W1"); b1_v = view("b1")
            W2_sb = view("W2"); b2_v = view("b2")
            Wh1_sb = view("Wh1"); bh1_v = view("bh1")
            Wh2_sb = view("Wh2"); bh2_v = view("bh2")
            srow0 = view("xsown").rearrange("p (w c) -> p w c", c=HID)

            # layer-2 gather table (h1s rows), own shard + AllGathered in
            # four quarter-major chunks so the collective overlaps layer-1
            # tail compute.  Raw dram tensors, manually registered in the
            # DGE table (SWDGE descriptor relocation needs it).
            tbl2_own = nc.dram_tensor("tbl2own", [cfg.PAD_NPC, HID], bf)
            tbl2q = [nc.dram_tensor(f"tbl2q{q}", [cfg.QROWS[q], HID], bf)
                     for q in range(4)]
            for t in (P_xs, *tbl2q):
                mloc = nc.lookup_mloc(t)
                if mloc.table_entry_id is None:
                    mloc.table_entry_id = len(nc.dge_table) + 1
                    nc.dge_table.append(mloc.name)

            def allgather_quarter(q):
                lo0 = cfg.QW0[q] * 128
                hi0 = (cfg.QW0[q] + cfg.QWIN[q]) * 128
                if no_coll:
                    nc.sync.dma_start(tbl2q[q][0:hi0 - lo0, :],
                                      tbl2_own[lo0:hi0, :])
                else:
                    nc.gpsimd.collective_compute(
                        "AllGather", mybir.AluOpType.bypass,
                        ins=[tbl2_own[lo0:hi0, :]], outs=[tbl2q[q][:]],
                        replica_groups=replica)

            ps_pool = psg.tile([128, G], f32, space="PSUM", tag="g")

            # layer-2 self rows: written in place by layer-1 window finish
            srow1 = cp.tile([128, NW, 128], bf, tag="c_srow1", name="c_srow1")

            wslots = {}

            def mm(layer, w, lhsT, rhs, remaining, full):
                first = remaining[w] == full[w]
                last = remaining[w] == 1
                remaining[w] -= 1
                nc.tensor.matmul(out=wslots[w][:], lhsT=lhsT, rhs=rhs,
                                 start=first, stop=False,
                                 skip_group_check=True)
                if last:
                    finish_window(layer, w, wslots[w][:])

            # ---- aggregation sweep (shared for both layers)
            def agg_layer(layer):
                q_end_sts = []
                acc = 0
                for q in range(4):
                    acc += cfg.QSTS[q]
                    q_end_sts.append(acc - 1)
                for st in range(NST):
                    sm = meta["st_meta"][st]
                    remaining = dict(sm["totals"])
                    full = sm["totals"]
                    wslots.clear()
                    wslots.update({w: psw.tile([128, 128], f32, space="PSUM",
                                               tag="win",
                                               name=f"wt_{layer}_{w}")
                                   for w in range(st * ST_W,
                                                  min((st + 1) * ST_W, NW))})
                    m_tiles = {}
                    for b in range(NBLK):
                        off = int(meta["seg_off"][st, b])
                        ln = int(meta["seg_len"][st, b])
                        if ln == 0:
                            continue
                        mt = mp.tile([128, ln // 128, 128], bf, tag="mtile",
                                     name=f"m_{layer}_{st}_{b}")
                        m_tiles[b] = (mt, off // 128)
                        if no_gather:
                            nc.gpsimd.memset(mt[:], 0.5)
                        else:
                            tsl = (P_xs[cfg.QROW0[b]:cfg.QROW0[b + 1], :]
                                   if layer == 0 else tbl2q[b][:])
                            nc.gpsimd.dma_gather(
                                mt[:], tsl,
                                idx_sb[:, off // 16: off // 16 + ln // 16],
                                ln, ln, HID,
                                single_packet=False, queue_num=b % 4)
                    for b in range(NBLK):
                        if b not in m_tiles:
                            continue
                        mt, mbase = m_tiles[b]
                        for (mode, gci0, gl, mms) in sm["groups"][b]:
                            if mode == "int":
                                w = mms[0][1]
                                p = w % 2
                                sq = sp.tile([128, max(gl, 1), 128], bf,
                                             tag="s",
                                             name=f"s_{layer}_{st}_{gci0}")
                                nc.vector.tensor_tensor(
                                    out=sq[:],
                                    in0=dstb_sb[:, gci0:gci0 + gl].to_broadcast(
                                        [128, gl, 128]),
                                    in1=iota256_sb[:, p * 128:
                                                   p * 128 + gl * 256].rearrange(
                                        "p (c j) -> p c j", j=256)[:, :, 0:128],
                                    op=mybir.AluOpType.is_equal)
                                for k in range(gl):
                                    mm(layer, w, mt[:, gci0 + k - mbase, :],
                                       sq[:, k, :], remaining, full)
                            else:
                                gci = gci0
                                sq = sp.tile([128, 2, 128], bf, tag="s",
                                             name=f"s_{layer}_{st}_{gci}_b")
                                nc.vector.tensor_tensor(
                                    out=sq[:],
                                    in0=dstb_sb[:, gci:gci + 1].to_broadcast(
                                        [128, 2, 128]),
                                    in1=iota256_sb[:, 0:256].rearrange(
                                        "p (c j) -> p c j", j=128),
                                    op=mybir.AluOpType.is_equal)
                                for (k, w) in mms:
                                    mm(layer, w, mt[:, gci - mbase, :],
                                       sq[:, k, :], remaining, full)
                    if layer == 0 and st in q_end_sts:
                        allgather_quarter(q_end_sts.index(st))

            def finish_window(layer, w, ps_w):
                # ps_w: [ch_in, dst] aggregate of gathered rows (carrying
                # inv_sqrt[src]); add the self-loop rows (own window, via
                # identity matmul transpose-accumulate), then evac applies
                # inv_sqrt[dst]; the layer weight + bias + relu run per
                # window in [dst, ch] layout.
                w0 = w * 128
                nc.tensor.matmul(out=ps_w,
                                 lhsT=(srow0 if layer == 0
                                       else srow1[:])[:, w, :],
                                 rhs=ident_sb[:],
                                 start=False, stop=True,
                                 skip_group_check=True)
                aggs = ep.tile([128, 128], bf, tag="aggs")
                nc.vector.tensor_tensor(out=aggs[:], in0=ps_w[:],
                                        in1=invb_sb[:, w0:w0 + 128],
                                        op=mybir.AluOpType.mult)
                W_sb = W1_sb if layer == 0 else W2_sb
                bias = b1_v if layer == 0 else b2_v
                psz = psa.tile([128, HID], f32, space="PSUM", tag="a")
                nc.tensor.matmul(out=psz[:], lhsT=aggs[:], rhs=W_sb[:],
                                 start=True, stop=False)
                nc.tensor.matmul(out=psz[:], lhsT=ones1b_v[0:1, :],
                                 rhs=bias[0:1, :], start=False, stop=True)
                if layer == 0:
                    # h1s = inv*relu(z): write straight into the layer-2
                    # self-row SBUF tile, then DMA that slice to the table
                    t2 = srow1[:][:, w, :]
                    nc.scalar.activation(t2, psz[:],
                                         mybir.ActivationFunctionType.Relu,
                                         scale=invs_sb[:, w:w + 1])
                    nc.sync.dma_start(tbl2_own[w0:w0 + 128, :], t2)
                else:
                    # h2 window [node, ch] -> pooled one-hot accumulate
                    h2w = ep.tile([128, HID], bf, tag="h2w")
                    nc.scalar.activation(h2w[:], psz[:],
                                         mybir.ActivationFunctionType.Relu)
                    sg = sp.tile([128, G], bf, tag="sg")
                    nc.vector.tensor_tensor(
                        out=sg[:],
                        in0=batch_sb[:, w:w + 1].to_broadcast([128, G]),
                        in1=iotag_sb[:],
                        op=mybir.AluOpType.is_equal)
                    nc.tensor.matmul(out=ps_pool[:], lhsT=h2w[:], rhs=sg[:],
                                     start=(w == 0), stop=(w == NW - 1),
                                     skip_group_check=True)

            agg_layer(0)
            agg_layer(1)

            # ---- pooled mean + head (replicated on every core)
            pooled_l = ep.tile([128, G], f32, tag="pool")
            nc.vector.tensor_copy(out=pooled_l[:], in_=ps_pool[:])
            ar_in = dp.tile([128, G], f32, tag="arin")
            ar_out = dp.tile([128, G], f32, addr_space="Shared", tag="arout")
            nc.sync.dma_start(ar_in[:], pooled_l[:])
            if no_coll:
                nc.sync.dma_start(ar_out[:], ar_in[:])
            else:
                nc.gpsimd.collective_compute(
                    "AllReduce", mybir.AluOpType.add,
                    ins=[ar_in.opt()], outs=[ar_out.opt()],
                    replica_groups=replica)
            pooled = ep.tile([128, G], f32, tag="pool")
            nc.sync.dma_start(pooled[:], ar_out[:])

            psc = psg.tile([128, G], f32, space="PSUM", tag="g")
            nc.tensor.matmul(out=psc[:], lhsT=ones1_v[0:1, :],
                             rhs=cnt_v[0:1, :], start=True, stop=True)
            cntb = ep.tile([128, G], f32, tag="pool")
            nc.vector.tensor_scalar_max(out=cntb[:], in0=psc[:], scalar1=1.0)
            invc = ep.tile([128, G], f32, tag="pool")
            nc.vector.reciprocal(invc[:], cntb[:])
            pmean = ep.tile([128, G], f32, tag="pool")
            nc.vector.tensor_tensor(out=pmean[:], in0=pooled[:], in1=invc[:],
                                    op=mybir.AluOpType.mult)

            psh1 = psg.tile([128, G], f32, space="PSUM", tag="g")
            nc.tensor.matmul(out=psh1[:], lhsT=Wh1_sb[:], rhs=pmean[:],
                             start=True, stop=True)
            relu1 = ep.tile([128, G], f32, tag="pool")
            nc.scalar.activation(relu1[:], psh1[:],
                                 mybir.ActivationFunctionType.Relu,
                                 bias=bh1_v[:, 0:1])
            psh2 = psg.tile([cfg.OUT, G], f32, space="PSUM", tag="g")
            nc.tensor.matmul(out=psh2[:], lhsT=Wh2_sb[:], rhs=relu1[:],
                             start=True, stop=True)
            out_sb = ep.tile([cfg.OUT, G], f32, tag="out")
            nc.vector.tensor_scalar_add(out=out_sb[:], in0=psh2[:],
                                        scalar1=bh2_v[0:cfg.OUT, 0:1])
            nc.sync.dma_start(t_out[:], out_sb[:])

    nc.compile()
    return nc


# ----------------------------------------------------------------- entry
def _run(inputs, cfg=CFG, trace=False):
    in_maps, meta = _host_prep(cfg, inputs["x"], inputs["edge_index"],
                               inputs["batch"])
    _add_weights(cfg, in_maps,
                 inputs["W1"], inputs["b1"], inputs["W2"], inputs["b2"],
                 inputs["Wh1"], inputs["bh1"], inputs["Wh2"], inputs["bh2"])
    nc = _build(cfg, meta)
    res = run_bass_kernel_spmd(nc, in_maps, list(range(cfg.C)), trace=trace)
    out = np.ascontiguousarray(np.asarray(res.results[0]["out"]).T)
    return out, res


def kernel(**inputs) -> np.ndarray:
    out, _ = _run(inputs, CFG, trace=False)
    return out


# revision 21
# speedup vs baseline: 1.6125x; 1.0525x over previous
"""Trainium2 Bass kernel for DemoGraphNet (2-layer GCN + mean-pool + MLP head).

Self-contained: hardcodes problem shapes and the 8-core sharding strategy.

v7 = v5 aggregation pipeline + quarter-chunked overlapped AllGather +
single packed constant blob (2 input args total).

Late-weight: aggregation commutes with the layer weight matmul
(A_hat @ (X W) = (A_hat @ X) W), so each layer gathers RAW feature rows
(layer 1: xs = D^-1/2 x from a replicated bf16 input table; layer 2: h1s
rows AllGathered per quarter) and applies the 128x128 weight + bias +
relu per 128-node dst window after aggregation in [dst, ch] orientation.

Quarter-chunked AllGather: the gather-table row space is QUARTER-major
(4 supertile-aligned window groups, each <= 32768 rows so int16 gather
indices still reach them).  Each quarter AllGathers as soon as layer 1
finishes its windows, overlapping the collective with the layer-1 tail;
layer-2 gather blocks == quarters, so only the last quarter's collective
can stall the layer-2 pipeline (~1/4 of the old exposure).

Packed blob: per-exec dispatch cost on this runtime scales with the
argument count (~70us/arg), dwarfing most device-side effects, so every
constant (gather indices, dst codes, one-hot iota bands, inv-sqrt
tables, weights, the layer-1 self-row image) is packed host-side into
one [128, B] uint8 image, DMA'd once, and sliced/bitcast on SBUF.

Edges live on their dst core (self-loops via identity matmul).
dma_gather (SWDGE) fetches 256B rows (bucket-sorted ascending for HBM
row locality) into [edge x chan] SBUF tiles.  Gathered rows carry
inv_sqrt[src]; window finish multiplies inv_sqrt[dst].  Pooled
per-graph sums are AllReduced; the tiny MLP head runs replicated.
"""

import math
import os
import sys

sys.path.insert(0, "/opt/trn_rl_repo")

import numpy as np
import ml_dtypes

import concourse.bass as bass
import concourse.mybir as mybir
import concourse.tile as tile
import concourse.bacc as bacc
from concourse import library_config
from concourse.bass_utils import run_bass_kernel_spmd

BF16 = ml_dtypes.bfloat16
F32 = np.float32


class Cfg:
    def __init__(self, n_nodes, n_graphs, st_w, n_cores=8, hid=128, out_c=8):
        assert n_nodes % n_cores == 0
        self.N = n_nodes
        self.G = n_graphs
        self.C = n_cores
        self.HID = hid
        self.OUT = out_c
        self.NPC = n_nodes // n_cores          # nodes per core
        self.WIN = 128                          # dst window width
        self.NW = math.ceil(self.NPC / 128)     # windows per core
        self.PAD_NPC = self.NW * 128
        self.ST_W = st_w                        # windows per supertile
        self.NST = math.ceil(self.NW / st_w)
        self.TBL_N = n_cores * self.PAD_NPC     # padded table rows
        self.NBLK = 4
        # split the NST supertiles into 4 quarters (last the smallest);
        # table layout is quarter-major so each quarter AllGathers
        # independently and is a contiguous int16-indexable gather block
        base = self.NST // 4
        rem = self.NST - 4 * base
        self.QSTS = [base + (1 if i < rem else 0) for i in range(4)]
        self.QW0 = [0]
        for i in range(4):
            self.QW0.append(min(self.QW0[-1] + self.QSTS[i] * st_w, self.NW))
        self.QWIN = [self.QW0[i + 1] - self.QW0[i] for i in range(4)]
        self.QROWS = [q * 128 * n_cores for q in self.QWIN]
        self.QROW0 = [0]
        for i in range(4):
            self.QROW0.append(self.QROW0[-1] + self.QROWS[i])
        assert all(r <= 32768 for r in self.QROWS), self.QROWS
        self.WQ_OF_W = []
        for i in range(4):
            self.WQ_OF_W += [i] * self.QWIN[i]


CFG = Cfg(n_nodes=100000, n_graphs=256, st_w=4)


# ------------------------------------------------------- blob layout
def _blob_layout(cfg, EP, NCH):
    """Column-byte layout of the packed constant image [128, total]."""
    sec = [
        ("idx",     np.int16, EP // 16),
        ("identb",  BF16,     128),
        ("dstcb",   BF16,     NCH),
        ("invb",    BF16,     cfg.PAD_NPC),
        ("invpp",   F32,      cfg.NW),
        ("batchpp", F32,      cfg.NW),
        ("iota256", BF16,     1280),
        ("iotag",   BF16,     cfg.G),
        ("cnt",     F32,      cfg.G),
        ("ones1",   F32,      128),
        ("ones1b",  BF16,     128),
        ("W1",      BF16,     cfg.HID),
        ("b1",      BF16,     cfg.HID),
        ("W2",      BF16,     cfg.HID),
        ("b2",      BF16,     cfg.HID),
        ("Wh1",     F32,      cfg.HID),
        ("bh1",     F32,      1),
        ("Wh2",     F32,      cfg.OUT),
        ("bh2",     F32,      1),
        ("xsown",   BF16,     cfg.NW * cfg.HID),
    ]
    layout = {}
    off = 0
    for name, dt, cols in sec:
        nbytes = cols * np.dtype(dt).itemsize
        layout[name] = (off, dt, cols)
        off += (nbytes + 3) // 4 * 4
    return layout, off


def _pack_blob(cfg, EP, NCH, pieces):
    layout, total = _blob_layout(cfg, EP, NCH)
    blob = np.zeros((128, total), dtype=np.uint8)
    for name, (off, dt, cols) in layout.items():
        arr = np.ascontiguousarray(pieces[name].astype(dt))
        assert arr.ndim == 2 and arr.shape[1] == cols, (name, arr.shape, cols)
        p = arr.shape[0]
        blob[0:p, off:off + cols * np.dtype(dt).itemsize] = arr.view(np.uint8)
    return blob


# ----------------------------------------------------------------- host prep
def _host_prep(cfg, x, edge_index, batch):
    """Shard + marshal inputs. Index bookkeeping plus input re-encoding
    (node->slot placement, scaling x rows by inv_sqrt and casting to bf16);
    every matmul and all aggregation FLOPs run on device."""
    N, C = cfg.N, cfg.C
    NPC, WIN, NW, ST_W, NST = cfg.NPC, cfg.WIN, cfg.NW, cfg.ST_W, cfg.NST
    NBLK, PAD_NPC = cfg.NBLK, cfg.PAD_NPC
    TBL_N = cfg.TBL_N

    src = np.asarray(edge_index[0], dtype=np.int64)
    dst = np.asarray(edge_index[1], dtype=np.int64)
    batch = np.asarray(batch, dtype=np.int64)
    x = np.asarray(x, dtype=np.float32)

    deg = (np.bincount(dst, minlength=N) + 1).astype(np.float64)
    inv_sqrt = (1.0 / np.sqrt(deg)).astype(np.float32)

    # ---- balanced node -> slot placement.  Pass 1: degree-sorted snake
    # over all (core, window) slots equalizes window totals.  Pass 2:
    # within each src-block's quarter, re-deal nodes to windows matching
    # per-block in-degree vectors to the per-round target, so per-(window,
    # src-block) bucket counts are near-uniform across cores too.
    NWT = TBL_N // WIN                      # total windows (all cores)
    order = np.argsort(-deg, kind="stable")
    slot_of_node = np.empty(N, dtype=np.int64)
    nfull = N // NWT
    for r in range(nfull):
        nodes_r = order[r * NWT:(r + 1) * NWT]
        wins = np.arange(NWT) if r % 2 == 0 else np.arange(NWT)[::-1]
        slot_of_node[nodes_r] = wins * WIN + r
    rem = N - nfull * NWT
    if rem:
        nodes_r = order[nfull * NWT:]
        wins = np.arange(rem) if nfull % 2 == 0 else NWT - 1 - np.arange(rem)
        slot_of_node[nodes_r] = wins * WIN + nfull

    # per-node per-src-block in-degree under pass-1 source placement
    # (pass 2 keeps every node inside its quarter, so src blocks are stable)
    wq_of_w = np.asarray(cfg.WQ_OF_W, dtype=np.int64)   # local window -> blk
    src_blk1 = wq_of_w[(slot_of_node[src] // WIN) % NW]
    dvec = np.zeros((N, NBLK), dtype=np.int64)
    np.add.at(dvec, (dst, src_blk1), 1)

    gwin_all = np.arange(TBL_N // WIN)
    for q in range(NBLK):
        qwins = gwin_all[wq_of_w[gwin_all % NW] == q]
        wpq = len(qwins)
        qnodes = np.nonzero(
            wq_of_w[(slot_of_node // WIN) % NW] == q)[0]
        qn = qnodes[np.argsort(-deg[qnodes], kind="stable")]
        dv = dvec[qn].astype(np.float64)            # [nq, NBLK]
        nq = len(qn)
        S = np.zeros((wpq, NBLK))
        fill = np.zeros(wpq, dtype=np.int64)
        new_slot = np.empty(nq, dtype=np.int64)
        pos = 0
        rnd = 0
        while pos < nq:
            take = min(wpq, nq - pos)
            rn = np.arange(pos, pos + take)
            avail = np.ones(wpq, dtype=bool)
            avail[fill >= WIN] = False
            # deficit-matching: most-imbalanced nodes first, window with
            # the largest deficit in the node's dominant block
            node_order = rn[np.argsort(-np.abs(dv[rn] - dv[rn].mean(0)).max(1))]
            target = S[avail].mean(0) if avail.any() else S.mean(0)
            D = (target[None, :] - S)                # deficit per window
            for i in node_order:
                b = int(np.argmax(dv[i] - dv[rn].mean(0)))
                cand = np.nonzero(avail)[0]
                wsel = cand[np.argmax(D[cand, b])]
                new_slot[i] = qwins[wsel] * WIN + fill[wsel]
                S[wsel] += dv[i]
                D[wsel] -= dv[i]
                fill[wsel] += 1
                avail[wsel] = fill[wsel] < WIN
            pos += take
            rnd += 1
        slot_of_node[qn] = new_slot

    node_of_slot = np.full(TBL_N, -1, dtype=np.int64)
    node_of_slot[slot_of_node] = np.arange(N)
    valid_slot = node_of_slot >= 0

    # quarter-major table row for every (old-numbering) global slot
    g_of_s = np.arange(TBL_N) // WIN
    i_of_s = np.arange(TBL_N) % WIN
    c_of_s = g_of_s // NW
    w_of_s = g_of_s % NW
    q_of_s = wq_of_w[w_of_s]
    qw0 = np.asarray(cfg.QW0[:4], dtype=np.int64)
    qwin = np.asarray(cfg.QWIN, dtype=np.int64)
    qrow0 = np.asarray(cfg.QROW0[:4], dtype=np.int64)
    tablerow_of_slot = (qrow0[q_of_s] + c_of_s * qwin[q_of_s] * WIN
                        + (w_of_s - qw0[q_of_s]) * WIN + i_of_s)

    # self-loops (the +I in A_hat) are NOT streamed as edges: each window
    # finish adds its own rows via an identity matmul instead (keeps the
    # per-(window, src-block) bucket counts balanced across cores).
    src_slot = slot_of_node[src]
    dst_slotg = slot_of_node[dst]
    core = dst_slotg // PAD_NPC
    tblrow = tablerow_of_slot[src_slot]
    blk = wq_of_w[(src_slot // WIN) % NW]
    w_loc = (dst_slotg % PAD_NPC) // WIN
    slot_in_win = dst_slotg % WIN

    # bucket quotas: max count over cores, rounded to 16
    key = (core * NW + w_loc) * NBLK + blk
    counts = np.bincount(key, minlength=C * NW * NBLK).reshape(C, NW, NBLK)
    quota = counts.max(axis=0)
    quota = ((quota + 15) // 16) * 16          # [NW, NBLK]

    # segment layout: stream ordered (st, blk, w); segments pad to 128
    bucket_base = np.zeros((NW, NBLK), dtype=np.int64)
    seg_off = np.zeros((NST, NBLK), dtype=np.int64)
    seg_len = np.zeros((NST, NBLK), dtype=np.int64)
    st_meta = []                                # per st: emission metadata
    pos = 0
    for st in range(NST):
        ws = list(range(st * ST_W, min((st + 1) * ST_W, NW)))
        seg_groups = {}
        for b in range(NBLK):
            seg_off[st, b] = pos
            bounds = []
            p0 = pos
            for w in ws:
                bucket_base[w, b] = pos
                bounds.append((w, pos - p0, pos - p0 + quota[w, b]))
                pos += quota[w, b]
            L = (pos - p0 + 127) // 128 * 128
            pos = p0 + L
            seg_len[st, b] = L
            gci0 = p0 // 128
            # classify chunks: interior (one window) vs boundary (two)
            groups = []          # (mode, gci, gl, [(k_or_band, w)...])
            k = 0
            nch_seg = L // 128
            while k < nch_seg:
                lo, hi = 128 * k, 128 * (k + 1)
                ov = [w for (w, s, e) in bounds if s < hi and lo < e]
                assert 1 <= len(ov) <= 2, (st, b, k, ov)
                if len(ov) == 2:
                    weven = ov[0] if ov[0] % 2 == 0 else ov[1]
                    wodd = ov[1] if ov[1] % 2 == 1 else ov[0]
                    assert weven % 2 == 0 and wodd % 2 == 1
                    groups.append(("bnd", gci0 + k, 1,
                                   [(0, weven), (1, wodd)]))
                    k += 1
                else:
                    w = ov[0]
                    j = k
                    while j < nch_seg and j - k < 4:
                        lo2, hi2 = 128 * j, 128 * (j + 1)
                        ov2 = [ww for (ww, s, e) in bounds
                               if s < hi2 and lo2 < e]
                        if ov2 != [w]:
                            break
                        j += 1
                    groups.append(("int", gci0 + k, j - k,
                                   [(kk, w) for kk in range(j - k)]))
                    k = j
            seg_groups[b] = groups
        # first/last matmul flags per window within this st
        totals = {}
        for b in range(NBLK):
            for (_, _, _, mms) in seg_groups[b]:
                for (_, w) in mms:
                    totals[w] = totals.get(w, 0) + 1
        st_meta.append(dict(groups=seg_groups, totals=totals))
    EP = pos
    NCH = EP // 128

    cnt = np.bincount(batch, minlength=cfg.G).astype(np.float32)

    # replicated layer-1 gather table: xs = inv_sqrt * x, quarter-major rows
    xs = (x * inv_sqrt[:, None]).astype(BF16)
    xs_slot = np.zeros((TBL_N, cfg.HID), dtype=BF16)
    xs_slot[valid_slot] = xs[node_of_slot[valid_slot]]
    xs_pad = np.zeros((TBL_N, cfg.HID), dtype=BF16)
    xs_pad[tablerow_of_slot] = xs_slot

    inv_slot = np.ones(TBL_N, dtype=np.float32)
    inv_slot[valid_slot] = inv_sqrt[node_of_slot[valid_slot]]
    batch_slot = np.full(TBL_N, -1.0, dtype=np.float32)
    batch_slot[valid_slot] = batch[node_of_slot[valid_slot]].astype(np.float32)

    in_maps = []
    okey = ((w_loc // ST_W) * NBLK + blk) * NW + w_loc
    for c in range(C):
        sel = np.nonzero(core == c)[0]
        e_okey = okey[sel]
        e_row = tblrow[sel]
        order_e = np.lexsort((e_row, e_okey))
        sel = sel[order_e]
        # rank within bucket
        bkey = (w_loc[sel] * NBLK + blk[sel])
        change = np.ones(len(sel), dtype=bool)
        change[1:] = bkey[1:] != bkey[:-1]
        gstart = np.maximum.accumulate(np.where(change, np.arange(len(sel)), 0))
        rank = np.arange(len(sel)) - gstart
        posn = bucket_base[w_loc[sel], blk[sel]] + rank

        e_idx16 = np.zeros(EP, dtype=np.int16)
        e_dst = np.full(EP, -1.0, dtype=np.float32)
        e_idx16[posn] = (tblrow[sel] - qrow0[blk[sel]]).astype(np.int16)
        e_dst[posn] = (slot_in_win[sel]
                       + WIN * (w_loc[sel] % 2)).astype(np.float32)

        idx_img = np.tile(e_idx16.reshape(-1, 16).T, (8, 1)).copy()
        dst_colb = np.ascontiguousarray(e_dst.reshape(NCH, 128).T).astype(BF16)

        lo = c * PAD_NPC
        inv_own = inv_slot[lo:lo + PAD_NPC]
        inv_pp = inv_own.reshape(NW, 128).T.copy()
        inv_b = np.broadcast_to(inv_own, (128, PAD_NPC)).astype(BF16).copy()
        batch_pp = batch_slot[lo:lo + PAD_NPC].reshape(NW, 128).T.copy()
        # layer-1 self rows, pre-arranged [128 slot-in-window, NW*HID]
        xsown_img = (xs_slot[lo:lo + PAD_NPC]
                     .reshape(NW, 128, cfg.HID).transpose(1, 0, 2)
                     .reshape(128, NW * cfg.HID).copy())

        in_maps.append({
            "_xs": xs_pad,
            "_pieces": {
                "idx": idx_img, "identb": np.eye(128, dtype=np.float32),
                "dstcb": dst_colb, "invb": inv_b, "invpp": inv_pp,
                "batchpp": batch_pp,
                "iota256": np.broadcast_to(
                    np.tile(np.arange(256, dtype=np.float32), 5),
                    (128, 1280)),
                "iotag": np.broadcast_to(
                    np.arange(cfg.G, dtype=np.float32), (128, cfg.G)),
                "cnt": cnt[None, :],
                "ones1": np.ones((1, 128), dtype=np.float32),
                "ones1b": np.ones((1, 128), dtype=np.float32),
                "xsown": xsown_img,
            },
        })

    meta = dict(EP=EP, NCH=NCH, seg_off=seg_off, seg_len=seg_len,
                st_meta=st_meta)
    return in_maps, meta


def _add_weights(cfg, in_maps, W1, b1, W2, b2, Wh1, bh1, Wh2, bh2):
    wts = {
        "W1": np.asarray(W1, F32), "b1": np.asarray(b1, F32).reshape(1, -1),
        "W2": np.asarray(W2, F32), "b2": np.asarray(b2, F32).reshape(1, -1),
        "Wh1": np.asarray(Wh1, F32),
        "bh1": np.asarray(bh1, F32).reshape(-1, 1),
        "Wh2": np.asarray(Wh2, F32),
        "bh2": np.asarray(bh2, F32).reshape(-1, 1),
    }
    for m in in_maps:
        pieces = m.pop("_pieces")
        pieces.update(wts)
        NCH = pieces["dstcb"].shape[1]
        EP = NCH * 128
        blob = _pack_blob(cfg, EP, NCH, pieces)
        bpad = (-blob.shape[1]) % 256
        if bpad:
            blob = np.concatenate(
                [blob, np.zeros((128, bpad), np.uint8)], axis=1)
        # blob bytes as extra bf16 table rows: partition-major chunks
        brows = blob.view(BF16).reshape(128 * (blob.shape[1] // 256), 128)
        xs_pad = m.pop("_xs")
        m["mega"] = np.concatenate([xs_pad, brows], axis=0)


# ------------------------------------------------------------- program build
def _build(cfg, meta):
    NW, ST_W, NST, NBLK = cfg.NW, cfg.ST_W, cfg.NST, cfg.NBLK
    NCH, EP = meta["NCH"], meta["EP"]
    HID, G = cfg.HID, cfg.G
    bf = mybir.dt.bfloat16
    f32 = mybir.dt.float32
    u8 = mybir.dt.uint8

    no_coll = os.environ.get("GNN_NO_COLL") == "1"
    no_gather = os.environ.get("GNN_NO_GATHER") == "1"
    nc = bacc.Bacc("TRN2", target_bir_lowering=False, debug=False,
                   num_devices=cfg.C, num_swdge_queues=4)

    layout, blob_bytes = _blob_layout(cfg, EP, NCH)
    blob_bytes += (-blob_bytes) % 256
    blob_rows = 128 * (blob_bytes // 256)
    P_mega = nc.declare_dram_parameter(
        "mega", [cfg.TBL_N + blob_rows, HID], bf, isOutput=False)
    t_out = nc.declare_dram_parameter("out", [cfg.OUT, G], f32, isOutput=True)

    replica = [list(range(cfg.C))]
    mdt = {np.int16: mybir.dt.int16, BF16: bf, F32: f32}

    with tile.TileContext(nc) as tc:
        with (
            tc.tile_pool(name="const", bufs=1) as cp,
            tc.tile_pool(name="mtiles",
                         bufs=int(os.environ.get("GNN_MPBUFS", "8"))) as mp,
            tc.tile_pool(name="stiles", bufs=12) as sp,
            tc.tile_pool(name="evac", bufs=6) as ep,
            tc.tile_pool(name="psw",
                         bufs=int(os.environ.get("GNN_PSWBUFS", "4")),
                         space="PSUM") as psw,
            tc.tile_pool(name="psa",
                         bufs=int(os.environ.get("GNN_PSABUFS", "3")),
                         space="PSUM") as psa,
            tc.tile_pool(name="psg", bufs=1, space="PSUM") as psg,
            tc.tile_pool(name="dram", bufs=1, space="DRAM") as dp,
        ):
            nc.gpsimd.load_library(library_config.mlp)

            # ---- the packed constants, one DMA from the mega tail
            blob_sb = cp.tile([128, blob_bytes], u8, tag="c_blob",
                              name="c_blob")
            nc.sync.dma_start(
                blob_sb[:],
                P_mega[:].bitcast(u8)[cfg.TBL_N:cfg.TBL_N + blob_rows, :]
                .rearrange("(p r) c -> p (r c)", p=128))

            def view(name):
                off, dt, cols = layout[name]
                nb = cols * np.dtype(dt).itemsize
                return blob_sb[:, off:off + nb].bitcast(mdt[dt])

            idx_sb = view("idx")
            ident_sb = view("identb")
            dstb_sb = view("dstcb")
            invb_sb = view("invb")
            invs_sb = view("invpp")
            batch_sb = view("batchpp")
            iota256_sb = view("iota256")
            iotag_sb = view("iotag")
            cnt_v = view("cnt")
            ones1_v = view("ones1")
            ones1b_v = view("ones1b")
            W1_sb = view("W1"); b1_v = view("b1")
            W2_sb = view("W2"); b2_v = view("b2")
            Wh1_sb = view("Wh1"); bh1_v = view("bh1")
            Wh2_sb = view("Wh2"); bh2_v = view("bh2")
            srow0 = view("xsown").rearrange("p (w c) -> p w c", c=HID)

            # layer-2 gather table (h1s rows), own shard + AllGathered in
            # four quarter-major chunks so the collective overlaps layer-1
            # tail compute.  Raw dram tensors, manually registered in the
            # DGE table (SWDGE descriptor relocation needs it).
            tbl2_own = nc.dram_tensor("tbl2own", [cfg.PAD_NPC, HID], bf)
            tbl2q = [nc.dram_tensor(f"tbl2q{q}", [cfg.QROWS[q], HID], bf)
                     for q in range(4)]
            for t in (P_mega, *tbl2q):
                mloc = nc.lookup_mloc(t)
                if mloc.table_entry_id is None:
                    mloc.table_entry_id = len(nc.dge_table) + 1
                    nc.dge_table.append(mloc.name)

            def allgather_quarter(q):
                lo0 = cfg.QW0[q] * 128
                hi0 = (cfg.QW0[q] + cfg.QWIN[q]) * 128
                if no_coll:
                    nc.sync.dma_start(tbl2q[q][0:hi0 - lo0, :],
                                      tbl2_own[lo0:hi0, :])
                else:
                    nc.gpsimd.collective_compute(
                        "AllGather", mybir.AluOpType.bypass,
                        ins=[tbl2_own[lo0:hi0, :]], outs=[tbl2q[q][:]],
                        replica_groups=replica)

            ps_pool = psg.tile([128, G], f32, space="PSUM", tag="g")

            # layer-2 self rows: written in place by layer-1 window finish
            srow1 = cp.tile([128, NW, 128], bf, tag="c_srow1", name="c_srow1")

            wslots = {}

            def mm(layer, w, lhsT, rhs, remaining, full):
                first = remaining[w] == full[w]
                last = remaining[w] == 1
                remaining[w] -= 1
                nc.tensor.matmul(out=wslots[w][:], lhsT=lhsT, rhs=rhs,
                                 start=first, stop=False,
                                 skip_group_check=True)
                if last:
                    finish_window(layer, w, wslots[w][:])

            # ---- aggregation sweep (shared for both layers)
            def agg_layer(layer):
                q_end_sts = []
                acc = 0
                for q in range(4):
                    acc += cfg.QSTS[q]
                    q_end_sts.append(acc - 1)
                for st in range(NST):
                    sm = meta["st_meta"][st]
                    remaining = dict(sm["totals"])
                    full = sm["totals"]
                    wslots.clear()
                    wslots.update({w: psw.tile([128, 128], f32, space="PSUM",
                                               tag="win",
                                               name=f"wt_{layer}_{w}")
                                   for w in range(st * ST_W,
                                                  min((st + 1) * ST_W, NW))})
                    m_tiles = {}
                    for b in range(NBLK):
                        off = int(meta["seg_off"][st, b])
                        ln = int(meta["seg_len"][st, b])
                        if ln == 0:
                            continue
                        mt = mp.tile([128, ln // 128, 128], bf, tag="mtile",
                                     name=f"m_{layer}_{st}_{b}")
                        m_tiles[b] = (mt, off // 128)
                        if no_gather:
                            nc.gpsimd.memset(mt[:], 0.5)
                        else:
                            tsl = (P_mega[cfg.QROW0[b]:
                                          cfg.QROW0[b + 1], :]
                                   if layer == 0 else tbl2q[b][:])
                            nc.gpsimd.dma_gather(
                                mt[:], tsl,
                                idx_sb[:, off // 16: off // 16 + ln // 16],
                                ln, ln, HID,
                                single_packet=False, queue_num=b % 4)
                    for b in range(NBLK):
                        if b not in m_tiles:
                            continue
                        mt, mbase = m_tiles[b]
                        for (mode, gci0, gl, mms) in sm["groups"][b]:
                            if mode == "int":
                                w = mms[0][1]
                                p = w % 2
                                sq = sp.tile([128, max(gl, 1), 128], bf,
                                             tag="s",
                                             name=f"s_{layer}_{st}_{gci0}")
                                nc.vector.tensor_tensor(
                                    out=sq[:],
                                    in0=dstb_sb[:, gci0:gci0 + gl].to_broadcast(
                                        [128, gl, 128]),
                                    in1=iota256_sb[:, p * 128:
                                                   p * 128 + gl * 256].rearrange(
                                        "p (c j) -> p c j", j=256)[:, :, 0:128],
                                    op=mybir.AluOpType.is_equal)
                                for k in range(gl):
                                    mm(layer, w, mt[:, gci0 + k - mbase, :],
                                       sq[:, k, :], remaining, full)
                            else:
                                gci = gci0
                                sq = sp.tile([128, 2, 128], bf, tag="s",
                                             name=f"s_{layer}_{st}_{gci}_b")
                                nc.vector.tensor_tensor(
                                    out=sq[:],
                                    in0=dstb_sb[:, gci:gci + 1].to_broadcast(
                                        [128, 2, 128]),
                                    in1=iota256_sb[:, 0:256].rearrange(
                                        "p (c j) -> p c j", j=128),
                                    op=mybir.AluOpType.is_equal)
                                for (k, w) in mms:
                                    mm(layer, w, mt[:, gci - mbase, :],
                                       sq[:, k, :], remaining, full)
                    if layer == 0 and st in q_end_sts:
                        allgather_quarter(q_end_sts.index(st))

            def finish_window(layer, w, ps_w):
                # ps_w: [ch_in, dst] aggregate of gathered rows (carrying
                # inv_sqrt[src]); add the self-loop rows (own window, via
                # identity matmul transpose-accumulate), then evac applies
                # inv_sqrt[dst]; the layer weight + bias + relu run per
                # window in [dst, ch] layout.
                w0 = w * 128
                nc.tensor.matmul(out=ps_w,
                                 lhsT=(srow0 if layer == 0
                                       else srow1[:])[:, w, :],
                                 rhs=ident_sb[:],
                                 start=False, stop=True,
                                 skip_group_check=True)
                aggs = ep.tile([128, 128], bf, tag="aggs")
                nc.vector.tensor_tensor(out=aggs[:], in0=ps_w[:],
                                        in1=invb_sb[:, w0:w0 + 128],
                                        op=mybir.AluOpType.mult)
                W_sb = W1_sb if layer == 0 else W2_sb
                bias = b1_v if layer == 0 else b2_v
                psz = psa.tile([128, HID], f32, space="PSUM", tag="a")
                nc.tensor.matmul(out=psz[:], lhsT=aggs[:], rhs=W_sb[:],
                                 start=True, stop=False)
                nc.tensor.matmul(out=psz[:], lhsT=ones1b_v[0:1, :],
                                 rhs=bias[0:1, :], start=False, stop=True)
                if layer == 0:
                    # h1s = inv*relu(z): write straight into the layer-2
                    # self-row SBUF tile, then DMA that slice to the table
                    t2 = srow1[:][:, w, :]
                    nc.scalar.activation(t2, psz[:],
                                         mybir.ActivationFunctionType.Relu,
                                         scale=invs_sb[:, w:w + 1])
                    nc.sync.dma_start(tbl2_own[w0:w0 + 128, :], t2)
                else:
                    # h2 window [node, ch] -> pooled one-hot accumulate
                    h2w = ep.tile([128, HID], bf, tag="h2w")
                    nc.scalar.activation(h2w[:], psz[:],
                                         mybir.ActivationFunctionType.Relu)
                    sg = sp.tile([128, G], bf, tag="sg")
                    nc.vector.tensor_tensor(
                        out=sg[:],
                        in0=batch_sb[:, w:w + 1].to_broadcast([128, G]),
                        in1=iotag_sb[:],
                        op=mybir.AluOpType.is_equal)
                    nc.tensor.matmul(out=ps_pool[:], lhsT=h2w[:], rhs=sg[:],
                                     start=(w == 0), stop=(w == NW - 1),
                                     skip_group_check=True)

            agg_layer(0)
            agg_layer(1)

            # ---- pooled mean + head (replicated on every core)
            pooled_l = ep.tile([128, G], f32, tag="pool")
            nc.vector.tensor_copy(out=pooled_l[:], in_=ps_pool[:])
            ar_in = dp.tile([128, G], f32, tag="arin")
            ar_out = dp.tile([128, G], f32, addr_space="Shared", tag="arout")
            nc.sync.dma_start(ar_in[:], pooled_l[:])
            if no_coll:
                nc.sync.dma_start(ar_out[:], ar_in[:])
            else:
                nc.gpsimd.collective_compute(
                    "AllReduce", mybir.AluOpType.add,
                    ins=[ar_in.opt()], outs=[ar_out.opt()],
                    replica_groups=replica)
            pooled = ep.tile([128, G], f32, tag="pool")
            nc.sync.dma_start(pooled[:], ar_out[:])

            psc = psg.tile([128, G], f32, space="PSUM", tag="g")
            nc.tensor.matmul(out=psc[:], lhsT=ones1_v[0:1, :],
                             rhs=cnt_v[0:1, :], start=True, stop=True)
            cntb = ep.tile([128, G], f32, tag="pool")
            nc.vector.tensor_scalar_max(out=cntb[:], in0=psc[:], scalar1=1.0)
            invc = ep.tile([128, G], f32, tag="pool")
            nc.vector.reciprocal(invc[:], cntb[:])
            pmean = ep.tile([128, G], f32, tag="pool")
            nc.vector.tensor_tensor(out=pmean[:], in0=pooled[:], in1=invc[:],
                                    op=mybir.AluOpType.mult)

            psh1 = psg.tile([128, G], f32, space="PSUM", tag="g")
            nc.tensor.matmul(out=psh1[:], lhsT=Wh1_sb[:], rhs=pmean[:],
                             start=True, stop=True)
            relu1 = ep.tile([128, G], f32, tag="pool")
            nc.scalar.activation(relu1[:], psh1[:],
                                 mybir.ActivationFunctionType.Relu,
                                 bias=bh1_v[:, 0:1])
            psh2 = psg.tile([cfg.OUT, G], f32, space="PSUM", tag="g")
            nc.tensor.matmul(out=psh2[:], lhsT=Wh2_sb[:], rhs=relu1[:],
                             start=True, stop=True)
            out_sb = ep.tile([cfg.OUT, G], f32, tag="out")
            nc.vector.tensor_scalar_add(out=out_sb[:], in0=psh2[:],
                                        scalar1=bh2_v[0:cfg.OUT, 0:1])
            nc.sync.dma_start(t_out[:], out_sb[:])

    nc.compile()
    return nc


# ----------------------------------------------------------------- entry
def _run(inputs, cfg=CFG, trace=False):
    in_maps, meta = _host_prep(cfg, inputs["x"], inputs["edge_index"],
                               inputs["batch"])
    _add_weights(cfg, in_maps,
                 inputs["W1"], inputs["b1"], inputs["W2"], inputs["b2"],
                 inputs["Wh1"], inputs["bh1"], inputs["Wh2"], inputs["bh2"])
    nc = _build(cfg, meta)
    res = run_bass_kernel_spmd(nc, in_maps, list(range(cfg.C)), trace=trace)
    out = np.ascontiguousarray(np.asarray(res.results[0]["out"]).T)
    return out, res


def kernel(**inputs) -> np.ndarray:
    out, _ = _run(inputs, CFG, trace=False)
    return out


# revision 23
# speedup vs baseline: 1.6241x; 1.0072x over previous
"""Trainium2 Bass kernel for DemoGraphNet (2-layer GCN + mean-pool + MLP head).

Self-contained: hardcodes problem shapes and the 8-core sharding strategy.

v7 = v5 aggregation pipeline + quarter-chunked overlapped AllGather +
single packed constant blob (2 input args total).

Late-weight: aggregation commutes with the layer weight matmul
(A_hat @ (X W) = (A_hat @ X) W), so each layer gathers RAW feature rows
(layer 1: xs = D^-1/2 x from a replicated bf16 input table; layer 2: h1s
rows AllGathered per quarter) and applies the 128x128 weight + bias +
relu per 128-node dst window after aggregation in [dst, ch] orientation.

Quarter-chunked AllGather: the gather-table row space is QUARTER-major
(4 supertile-aligned window groups, each <= 32768 rows so int16 gather
indices still reach them).  Each quarter AllGathers as soon as layer 1
finishes its windows, overlapping the collective with the layer-1 tail;
layer-2 gather blocks == quarters, so only the last quarter's collective
can stall the layer-2 pipeline (~1/4 of the old exposure).

Packed blob: per-exec dispatch cost on this runtime scales with the
argument count (~70us/arg), dwarfing most device-side effects, so every
constant (gather indices, dst codes, one-hot iota bands, inv-sqrt
tables, weights, the layer-1 self-row image) is packed host-side into
one [128, B] uint8 image, DMA'd once, and sliced/bitcast on SBUF.

Edges live on their dst core (self-loops via identity matmul).
dma_gather (SWDGE) fetches 256B rows (bucket-sorted ascending for HBM
row locality) into [edge x chan] SBUF tiles.  Gathered rows carry
inv_sqrt[src]; window finish multiplies inv_sqrt[dst].  Pooled
per-graph sums are AllReduced; the tiny MLP head runs replicated.
"""

import math
import os
import sys

sys.path.insert(0, "/opt/trn_rl_repo")

import numpy as np
import ml_dtypes

import concourse.bass as bass
import concourse.mybir as mybir
import concourse.tile as tile
import concourse.bacc as bacc
from concourse import library_config
from concourse.bass_utils import run_bass_kernel_spmd

BF16 = ml_dtypes.bfloat16
F32 = np.float32


class Cfg:
    def __init__(self, n_nodes, n_graphs, st_w, n_cores=8, hid=128, out_c=8):
        assert n_nodes % n_cores == 0
        self.N = n_nodes
        self.G = n_graphs
        self.C = n_cores
        self.HID = hid
        self.OUT = out_c
        self.NPC = n_nodes // n_cores          # nodes per core
        self.WIN = 128                          # dst window width
        self.NW = math.ceil(self.NPC / 128)     # windows per core
        self.PAD_NPC = self.NW * 128
        self.ST_W = st_w                        # windows per supertile
        self.NST = math.ceil(self.NW / st_w)
        self.TBL_N = n_cores * self.PAD_NPC     # padded table rows
        self.NBLK = 4
        # split the NST supertiles into 4 quarters (last the smallest);
        # table layout is quarter-major so each quarter AllGathers
        # independently and is a contiguous int16-indexable gather block
        base = self.NST // 4
        rem = self.NST - 4 * base
        self.QSTS = [base + (1 if i < rem else 0) for i in range(4)]
        self.QW0 = [0]
        for i in range(4):
            self.QW0.append(min(self.QW0[-1] + self.QSTS[i] * st_w, self.NW))
        self.QWIN = [self.QW0[i + 1] - self.QW0[i] for i in range(4)]
        self.QROWS = [q * 128 * n_cores for q in self.QWIN]
        self.QROW0 = [0]
        for i in range(4):
            self.QROW0.append(self.QROW0[-1] + self.QROWS[i])
        assert all(r <= 32768 for r in self.QROWS), self.QROWS
        self.WQ_OF_W = []
        for i in range(4):
            self.WQ_OF_W += [i] * self.QWIN[i]


CFG = Cfg(n_nodes=100000, n_graphs=256,
          st_w=int(os.environ.get("GNN_STW", "4")))


# ------------------------------------------------------- blob layout
def _blob_layout(cfg, EP, NCH):
    """Column-byte layout of the packed constant image [128, total]."""
    sec = [
        ("idx",     np.int16, EP // 16),
        ("identb",  BF16,     128),
        ("dstcb",   BF16,     NCH),
        ("invb",    BF16,     cfg.PAD_NPC),
        ("invpp",   F32,      cfg.NW),
        ("batchpp", F32,      cfg.NW),
        ("iota256", BF16,     1280),
        ("iotag",   BF16,     cfg.G),
        ("cnt",     F32,      cfg.G),
        ("ones1",   F32,      128),
        ("ones1b",  BF16,     128),
        ("W1",      BF16,     cfg.HID),
        ("b1",      BF16,     cfg.HID),
        ("W2",      BF16,     cfg.HID),
        ("b2",      BF16,     cfg.HID),
        ("Wh1",     F32,      cfg.HID),
        ("bh1",     F32,      1),
        ("Wh2",     F32,      cfg.OUT),
        ("bh2",     F32,      1),
        ("xsown",   BF16,     cfg.NW * cfg.HID),
    ]
    layout = {}
    off = 0
    for name, dt, cols in sec:
        nbytes = cols * np.dtype(dt).itemsize
        layout[name] = (off, dt, cols)
        pad = 256 if name == "idx" else 4
        off += (nbytes + pad - 1) // pad * pad
    return layout, off


def _pack_blob(cfg, EP, NCH, pieces):
    layout, total = _blob_layout(cfg, EP, NCH)
    blob = np.zeros((128, total), dtype=np.uint8)
    for name, (off, dt, cols) in layout.items():
        arr = np.ascontiguousarray(pieces[name].astype(dt))
        assert arr.ndim == 2 and arr.shape[1] == cols, (name, arr.shape, cols)
        p = arr.shape[0]
        blob[0:p, off:off + cols * np.dtype(dt).itemsize] = arr.view(np.uint8)
    return blob


# ----------------------------------------------------------------- host prep
def _host_prep(cfg, x, edge_index, batch):
    """Shard + marshal inputs. Index bookkeeping plus input re-encoding
    (node->slot placement, scaling x rows by inv_sqrt and casting to bf16);
    every matmul and all aggregation FLOPs run on device."""
    N, C = cfg.N, cfg.C
    NPC, WIN, NW, ST_W, NST = cfg.NPC, cfg.WIN, cfg.NW, cfg.ST_W, cfg.NST
    NBLK, PAD_NPC = cfg.NBLK, cfg.PAD_NPC
    TBL_N = cfg.TBL_N

    src = np.asarray(edge_index[0], dtype=np.int64)
    dst = np.asarray(edge_index[1], dtype=np.int64)
    batch = np.asarray(batch, dtype=np.int64)
    x = np.asarray(x, dtype=np.float32)

    deg = (np.bincount(dst, minlength=N) + 1).astype(np.float64)
    inv_sqrt = (1.0 / np.sqrt(deg)).astype(np.float32)

    # ---- balanced node -> slot placement.  Pass 1: degree-sorted snake
    # over all (core, window) slots equalizes window totals.  Pass 2:
    # within each src-block's quarter, re-deal nodes to windows matching
    # per-block in-degree vectors to the per-round target, so per-(window,
    # src-block) bucket counts are near-uniform across cores too.
    NWT = TBL_N // WIN                      # total windows (all cores)
    order = np.argsort(-deg, kind="stable")
    slot_of_node = np.empty(N, dtype=np.int64)
    nfull = N // NWT
    for r in range(nfull):
        nodes_r = order[r * NWT:(r + 1) * NWT]
        wins = np.arange(NWT) if r % 2 == 0 else np.arange(NWT)[::-1]
        slot_of_node[nodes_r] = wins * WIN + r
    rem = N - nfull * NWT
    if rem:
        nodes_r = order[nfull * NWT:]
        wins = np.arange(rem) if nfull % 2 == 0 else NWT - 1 - np.arange(rem)
        slot_of_node[nodes_r] = wins * WIN + nfull

    # per-node per-src-block in-degree under pass-1 source placement
    # (pass 2 keeps every node inside its quarter, so src blocks are stable)
    wq_of_w = np.asarray(cfg.WQ_OF_W, dtype=np.int64)   # local window -> blk
    src_blk1 = wq_of_w[(slot_of_node[src] // WIN) % NW]
    dvec = np.zeros((N, NBLK), dtype=np.int64)
    np.add.at(dvec, (dst, src_blk1), 1)

    gwin_all = np.arange(TBL_N // WIN)
    for q in range(NBLK):
        qwins = gwin_all[wq_of_w[gwin_all % NW] == q]
        wpq = len(qwins)
        qnodes = np.nonzero(
            wq_of_w[(slot_of_node // WIN) % NW] == q)[0]
        qn = qnodes[np.argsort(-deg[qnodes], kind="stable")]
        dv = dvec[qn].astype(np.float64)            # [nq, NBLK]
        nq = len(qn)
        S = np.zeros((wpq, NBLK))
        fill = np.zeros(wpq, dtype=np.int64)
        new_slot = np.empty(nq, dtype=np.int64)
        pos = 0
        rnd = 0
        while pos < nq:
            take = min(wpq, nq - pos)
            rn = np.arange(pos, pos + take)
            avail = np.ones(wpq, dtype=bool)
            avail[fill >= WIN] = False
            # deficit-matching: most-imbalanced nodes first, window with
            # the largest deficit in the node's dominant block
            node_order = rn[np.argsort(-np.abs(dv[rn] - dv[rn].mean(0)).max(1))]
            target = S[avail].mean(0) if avail.any() else S.mean(0)
            D = (target[None, :] - S)                # deficit per window
            for i in node_order:
                b = int(np.argmax(dv[i] - dv[rn].mean(0)))
                cand = np.nonzero(avail)[0]
                wsel = cand[np.argmax(D[cand, b])]
                new_slot[i] = qwins[wsel] * WIN + fill[wsel]
                S[wsel] += dv[i]
                D[wsel] -= dv[i]
                fill[wsel] += 1
                avail[wsel] = fill[wsel] < WIN
            pos += take
            rnd += 1
        slot_of_node[qn] = new_slot

    node_of_slot = np.full(TBL_N, -1, dtype=np.int64)
    node_of_slot[slot_of_node] = np.arange(N)
    valid_slot = node_of_slot >= 0

    # quarter-major table row for every (old-numbering) global slot
    g_of_s = np.arange(TBL_N) // WIN
    i_of_s = np.arange(TBL_N) % WIN
    c_of_s = g_of_s // NW
    w_of_s = g_of_s % NW
    q_of_s = wq_of_w[w_of_s]
    qw0 = np.asarray(cfg.QW0[:4], dtype=np.int64)
    qwin = np.asarray(cfg.QWIN, dtype=np.int64)
    qrow0 = np.asarray(cfg.QROW0[:4], dtype=np.int64)
    tablerow_of_slot = (qrow0[q_of_s] + c_of_s * qwin[q_of_s] * WIN
                        + (w_of_s - qw0[q_of_s]) * WIN + i_of_s)

    # self-loops (the +I in A_hat) are NOT streamed as edges: each window
    # finish adds its own rows via an identity matmul instead (keeps the
    # per-(window, src-block) bucket counts balanced across cores).
    src_slot = slot_of_node[src]
    dst_slotg = slot_of_node[dst]
    core = dst_slotg // PAD_NPC
    tblrow = tablerow_of_slot[src_slot]
    blk = wq_of_w[(src_slot // WIN) % NW]
    w_loc = (dst_slotg % PAD_NPC) // WIN
    slot_in_win = dst_slotg % WIN

    # bucket quotas: max count over cores, rounded to 16
    key = (core * NW + w_loc) * NBLK + blk
    counts = np.bincount(key, minlength=C * NW * NBLK).reshape(C, NW, NBLK)
    quota = counts.max(axis=0)
    quota = ((quota + 15) // 16) * 16          # [NW, NBLK]

    # segment layout: stream ordered (st, blk, w); segments pad to 128
    bucket_base = np.zeros((NW, NBLK), dtype=np.int64)
    seg_off = np.zeros((NST, NBLK), dtype=np.int64)
    seg_len = np.zeros((NST, NBLK), dtype=np.int64)
    st_meta = []                                # per st: emission metadata
    pos = 0
    for st in range(NST):
        ws = list(range(st * ST_W, min((st + 1) * ST_W, NW)))
        seg_groups = {}
        for b in range(NBLK):
            seg_off[st, b] = pos
            bounds = []
            p0 = pos
            for w in ws:
                bucket_base[w, b] = pos
                bounds.append((w, pos - p0, pos - p0 + quota[w, b]))
                pos += quota[w, b]
            L = (pos - p0 + 127) // 128 * 128
            pos = p0 + L
            seg_len[st, b] = L
            gci0 = p0 // 128
            # classify chunks: interior (one window) vs boundary (two)
            groups = []          # (mode, gci, gl, [(k_or_band, w)...])
            k = 0
            nch_seg = L // 128
            while k < nch_seg:
                lo, hi = 128 * k, 128 * (k + 1)
                ov = [w for (w, s, e) in bounds if s < hi and lo < e]
                assert 1 <= len(ov) <= 2, (st, b, k, ov)
                if len(ov) == 2:
                    weven = ov[0] if ov[0] % 2 == 0 else ov[1]
                    wodd = ov[1] if ov[1] % 2 == 1 else ov[0]
                    assert weven % 2 == 0 and wodd % 2 == 1
                    groups.append(("bnd", gci0 + k, 1,
                                   [(0, weven), (1, wodd)]))
                    k += 1
                else:
                    w = ov[0]
                    j = k
                    while j < nch_seg and j - k < 4:
                        lo2, hi2 = 128 * j, 128 * (j + 1)
                        ov2 = [ww for (ww, s, e) in bounds
                               if s < hi2 and lo2 < e]
                        if ov2 != [w]:
                            break
                        j += 1
                    groups.append(("int", gci0 + k, j - k,
                                   [(kk, w) for kk in range(j - k)]))
                    k = j
            seg_groups[b] = groups
        # first/last matmul flags per window within this st
        totals = {}
        for b in range(NBLK):
            for (_, _, _, mms) in seg_groups[b]:
                for (_, w) in mms:
                    totals[w] = totals.get(w, 0) + 1
        st_meta.append(dict(groups=seg_groups, totals=totals))
    EP = pos
    NCH = EP // 128

    cnt = np.bincount(batch, minlength=cfg.G).astype(np.float32)

    # replicated layer-1 gather table: xs = inv_sqrt * x, quarter-major rows
    xs = (x * inv_sqrt[:, None]).astype(BF16)
    xs_slot = np.zeros((TBL_N, cfg.HID), dtype=BF16)
    xs_slot[valid_slot] = xs[node_of_slot[valid_slot]]
    xs_pad = np.zeros((TBL_N, cfg.HID), dtype=BF16)
    xs_pad[tablerow_of_slot] = xs_slot

    inv_slot = np.ones(TBL_N, dtype=np.float32)
    inv_slot[valid_slot] = inv_sqrt[node_of_slot[valid_slot]]
    batch_slot = np.full(TBL_N, -1.0, dtype=np.float32)
    batch_slot[valid_slot] = batch[node_of_slot[valid_slot]].astype(np.float32)

    in_maps = []
    okey = ((w_loc // ST_W) * NBLK + blk) * NW + w_loc
    for c in range(C):
        sel = np.nonzero(core == c)[0]
        e_okey = okey[sel]
        e_row = tblrow[sel]
        order_e = np.lexsort((e_row, e_okey))
        sel = sel[order_e]
        # rank within bucket
        bkey = (w_loc[sel] * NBLK + blk[sel])
        change = np.ones(len(sel), dtype=bool)
        change[1:] = bkey[1:] != bkey[:-1]
        gstart = np.maximum.accumulate(np.where(change, np.arange(len(sel)), 0))
        rank = np.arange(len(sel)) - gstart
        posn = bucket_base[w_loc[sel], blk[sel]] + rank

        e_idx16 = np.zeros(EP, dtype=np.int16)
        e_dst = np.full(EP, -1.0, dtype=np.float32)
        e_idx16[posn] = (tblrow[sel] - qrow0[blk[sel]]).astype(np.int16)
        e_dst[posn] = (slot_in_win[sel]
                       + WIN * (w_loc[sel] % 2)).astype(np.float32)

        idx_img = np.tile(e_idx16.reshape(-1, 16).T, (8, 1)).copy()
        dst_colb = np.ascontiguousarray(e_dst.reshape(NCH, 128).T).astype(BF16)

        lo = c * PAD_NPC
        inv_own = inv_slot[lo:lo + PAD_NPC]
        inv_pp = inv_own.reshape(NW, 128).T.copy()
        inv_b = np.broadcast_to(inv_own, (128, PAD_NPC)).astype(BF16).copy()
        batch_pp = batch_slot[lo:lo + PAD_NPC].reshape(NW, 128).T.copy()
        # layer-1 self rows, pre-arranged [128 slot-in-window, NW*HID]
        xsown_img = (xs_slot[lo:lo + PAD_NPC]
                     .reshape(NW, 128, cfg.HID).transpose(1, 0, 2)
                     .reshape(128, NW * cfg.HID).copy())

        in_maps.append({
            "_xs": xs_pad,
            "_pieces": {
                "idx": idx_img, "identb": np.eye(128, dtype=np.float32),
                "dstcb": dst_colb, "invb": inv_b, "invpp": inv_pp,
                "batchpp": batch_pp,
                "iota256": np.broadcast_to(
                    np.tile(np.arange(256, dtype=np.float32), 5),
                    (128, 1280)),
                "iotag": np.broadcast_to(
                    np.arange(cfg.G, dtype=np.float32), (128, cfg.G)),
                "cnt": cnt[None, :],
                "ones1": np.ones((1, 128), dtype=np.float32),
                "ones1b": np.ones((1, 128), dtype=np.float32),
                "xsown": xsown_img,
            },
        })

    meta = dict(EP=EP, NCH=NCH, seg_off=seg_off, seg_len=seg_len,
                st_meta=st_meta)
    return in_maps, meta


def _add_weights(cfg, in_maps, W1, b1, W2, b2, Wh1, bh1, Wh2, bh2):
    wts = {
        "W1": np.asarray(W1, F32), "b1": np.asarray(b1, F32).reshape(1, -1),
        "W2": np.asarray(W2, F32), "b2": np.asarray(b2, F32).reshape(1, -1),
        "Wh1": np.asarray(Wh1, F32),
        "bh1": np.asarray(bh1, F32).reshape(-1, 1),
        "Wh2": np.asarray(Wh2, F32),
        "bh2": np.asarray(bh2, F32).reshape(-1, 1),
    }
    for m in in_maps:
        pieces = m.pop("_pieces")
        pieces.update(wts)
        NCH = pieces["dstcb"].shape[1]
        EP = NCH * 128
        blob = _pack_blob(cfg, EP, NCH, pieces)
        bpad = (-blob.shape[1]) % 256
        if bpad:
            blob = np.concatenate(
                [blob, np.zeros((128, bpad), np.uint8)], axis=1)
        # blob bytes as extra bf16 table rows: partition-major chunks
        brows = blob.view(BF16).reshape(128 * (blob.shape[1] // 256), 128)
        xs_pad = m.pop("_xs")
        m["mega"] = np.concatenate([xs_pad, brows], axis=0)


# ------------------------------------------------------------- program build
def _build(cfg, meta):
    NW, ST_W, NST, NBLK = cfg.NW, cfg.ST_W, cfg.NST, cfg.NBLK
    NCH, EP = meta["NCH"], meta["EP"]
    HID, G = cfg.HID, cfg.G
    bf = mybir.dt.bfloat16
    f32 = mybir.dt.float32
    u8 = mybir.dt.uint8

    no_coll = os.environ.get("GNN_NO_COLL") == "1"
    no_gather = os.environ.get("GNN_NO_GATHER") == "1"
    nc = bacc.Bacc("TRN2", target_bir_lowering=False, debug=False,
                   num_devices=cfg.C, num_swdge_queues=4)

    layout, blob_bytes = _blob_layout(cfg, EP, NCH)
    blob_bytes += (-blob_bytes) % 256
    blob_rows = 128 * (blob_bytes // 256)
    P_mega = nc.declare_dram_parameter(
        "mega", [cfg.TBL_N + blob_rows, HID], bf, isOutput=False)
    t_out = nc.declare_dram_parameter("out", [cfg.OUT, G], f32, isOutput=True)

    replica = [list(range(cfg.C))]
    mdt = {np.int16: mybir.dt.int16, BF16: bf, F32: f32}

    with tile.TileContext(nc) as tc:
        with (
            tc.tile_pool(name="const", bufs=1) as cp,
            tc.tile_pool(name="mtiles",
                         bufs=int(os.environ.get("GNN_MPBUFS", "8"))) as mp,
            tc.tile_pool(name="stiles", bufs=12) as sp,
            tc.tile_pool(name="evac", bufs=6) as ep,
            tc.tile_pool(name="psw",
                         bufs=int(os.environ.get("GNN_PSWBUFS", "4")),
                         space="PSUM") as psw,
            tc.tile_pool(name="psa",
                         bufs=int(os.environ.get("GNN_PSABUFS", "3")),
                         space="PSUM") as psa,
            tc.tile_pool(name="psg", bufs=1, space="PSUM") as psg,
            tc.tile_pool(name="dram", bufs=1, space="DRAM") as dp,
        ):
            nc.gpsimd.load_library(library_config.mlp)

            # ---- the packed constants: idx section DMA'd first so the
            # layer-1 gathers start without waiting for the full blob
            blob_sb = cp.tile([128, blob_bytes], u8, tag="c_blob",
                              name="c_blob")
            mega_u8 = (P_mega[:].bitcast(u8)
                       [cfg.TBL_N:cfg.TBL_N + blob_rows, :]
                       .rearrange("(p r) c -> p (r c)", p=128))
            idx_end = ((layout["idx"][2] * 2) + 255) // 256 * 256
            if os.environ.get("GNN_NO_BLOBSPLIT") == "1":
                nc.sync.dma_start(blob_sb[:], mega_u8)
            else:
                nc.sync.dma_start(blob_sb[:, 0:idx_end],
                                  mega_u8[:, 0:idx_end])
                nc.sync.dma_start(blob_sb[:, idx_end:blob_bytes],
                                  mega_u8[:, idx_end:blob_bytes])

            def view(name):
                off, dt, cols = layout[name]
                nb = cols * np.dtype(dt).itemsize
                return blob_sb[:, off:off + nb].bitcast(mdt[dt])

            idx_sb = view("idx")
            ident_sb = view("identb")
            dstb_sb = view("dstcb")
            invb_sb = view("invb")
            invs_sb = view("invpp")
            batch_sb = view("batchpp")
            iota256_sb = view("iota256")
            iotag_sb = view("iotag")
            cnt_v = view("cnt")
            ones1_v = view("ones1")
            ones1b_v = view("ones1b")
            W1_sb = view("W1"); b1_v = view("b1")
            W2_sb = view("W2"); b2_v = view("b2")
            Wh1_sb = view("Wh1"); bh1_v = view("bh1")
            Wh2_sb = view("Wh2"); bh2_v = view("bh2")
            srow0 = view("xsown").rearrange("p (w c) -> p w c", c=HID)

            # layer-2 gather table (h1s rows), own shard + AllGathered in
            # four quarter-major chunks so the collective overlaps layer-1
            # tail compute.  Raw dram tensors, manually registered in the
            # DGE table (SWDGE descriptor relocation needs it).
            tbl2_own = nc.dram_tensor("tbl2own", [cfg.PAD_NPC, HID], bf)
            tbl2q = [nc.dram_tensor(f"tbl2q{q}", [cfg.QROWS[q], HID], bf)
                     for q in range(4)]
            for t in (P_mega, *tbl2q):
                mloc = nc.lookup_mloc(t)
                if mloc.table_entry_id is None:
                    mloc.table_entry_id = len(nc.dge_table) + 1
                    nc.dge_table.append(mloc.name)

            def allgather_quarter(q):
                lo0 = cfg.QW0[q] * 128
                hi0 = (cfg.QW0[q] + cfg.QWIN[q]) * 128
                if no_coll:
                    nc.sync.dma_start(tbl2q[q][0:hi0 - lo0, :],
                                      tbl2_own[lo0:hi0, :])
                else:
                    nc.gpsimd.collective_compute(
                        "AllGather", mybir.AluOpType.bypass,
                        ins=[tbl2_own[lo0:hi0, :]], outs=[tbl2q[q][:]],
                        replica_groups=replica)

            ps_pool = psg.tile([128, G], f32, space="PSUM", tag="g")

            # layer-2 self rows: written in place by layer-1 window finish
            srow1 = cp.tile([128, NW, 128], bf, tag="c_srow1", name="c_srow1")

            wslots = {}

            def mm(layer, w, lhsT, rhs, remaining, full):
                first = remaining[w] == full[w]
                last = remaining[w] == 1
                remaining[w] -= 1
                nc.tensor.matmul(out=wslots[w][:], lhsT=lhsT, rhs=rhs,
                                 start=first, stop=False,
                                 skip_group_check=True)
                if last:
                    finish_window(layer, w, wslots[w][:])

            # ---- aggregation sweep (shared for both layers)
            def agg_layer(layer):
                q_end_sts = []
                acc = 0
                for q in range(4):
                    acc += cfg.QSTS[q]
                    q_end_sts.append(acc - 1)
                for st in range(NST):
                    sm = meta["st_meta"][st]
                    remaining = dict(sm["totals"])
                    full = sm["totals"]
                    wslots.clear()
                    wslots.update({w: psw.tile([128, 128], f32, space="PSUM",
                                               tag="win",
                                               name=f"wt_{layer}_{w}")
                                   for w in range(st * ST_W,
                                                  min((st + 1) * ST_W, NW))})
                    m_tiles = {}
                    for b in range(NBLK):
                        off = int(meta["seg_off"][st, b])
                        ln = int(meta["seg_len"][st, b])
                        if ln == 0:
                            continue
                        mt = mp.tile([128, ln // 128, 128], bf, tag="mtile",
                                     name=f"m_{layer}_{st}_{b}")
                        m_tiles[b] = (mt, off // 128)
                        if no_gather:
                            nc.gpsimd.memset(mt[:], 0.5)
                        else:
                            tsl = (P_mega[cfg.QROW0[b]:
                                          cfg.QROW0[b + 1], :]
                                   if layer == 0 else tbl2q[b][:])
                            nc.gpsimd.dma_gather(
                                mt[:], tsl,
                                idx_sb[:, off // 16: off // 16 + ln // 16],
                                ln, ln, HID,
                                single_packet=False, queue_num=b % 4)
                    for b in range(NBLK):
                        if b not in m_tiles:
                            continue
                        mt, mbase = m_tiles[b]
                        for (mode, gci0, gl, mms) in sm["groups"][b]:
                            if mode == "int":
                                w = mms[0][1]
                                p = w % 2
                                sq = sp.tile([128, max(gl, 1), 128], bf,
                                             tag="s",
                                             name=f"s_{layer}_{st}_{gci0}")
                                nc.vector.tensor_tensor(
                                    out=sq[:],
                                    in0=dstb_sb[:, gci0:gci0 + gl].to_broadcast(
                                        [128, gl, 128]),
                                    in1=iota256_sb[:, p * 128:
                                                   p * 128 + gl * 256].rearrange(
                                        "p (c j) -> p c j", j=256)[:, :, 0:128],
                                    op=mybir.AluOpType.is_equal)
                                for k in range(gl):
                                    mm(layer, w, mt[:, gci0 + k - mbase, :],
                                       sq[:, k, :], remaining, full)
                            else:
                                gci = gci0
                                sq = sp.tile([128, 2, 128], bf, tag="s",
                                             name=f"s_{layer}_{st}_{gci}_b")
                                nc.vector.tensor_tensor(
                                    out=sq[:],
                                    in0=dstb_sb[:, gci:gci + 1].to_broadcast(
                                        [128, 2, 128]),
                                    in1=iota256_sb[:, 0:256].rearrange(
                                        "p (c j) -> p c j", j=128),
                                    op=mybir.AluOpType.is_equal)
                                for (k, w) in mms:
                                    mm(layer, w, mt[:, gci - mbase, :],
                                       sq[:, k, :], remaining, full)
                    if layer == 0 and st in q_end_sts:
                        allgather_quarter(q_end_sts.index(st))

            def finish_window(layer, w, ps_w):
                # ps_w: [ch_in, dst] aggregate of gathered rows (carrying
                # inv_sqrt[src]); add the self-loop rows (own window, via
                # identity matmul transpose-accumulate), then evac applies
                # inv_sqrt[dst]; the layer weight + bias + relu run per
                # window in [dst, ch] layout.
                w0 = w * 128
                nc.tensor.matmul(out=ps_w,
                                 lhsT=(srow0 if layer == 0
                                       else srow1[:])[:, w, :],
                                 rhs=ident_sb[:],
                                 start=False, stop=True,
                                 skip_group_check=True)
                aggs = ep.tile([128, 128], bf, tag="aggs")
                nc.vector.tensor_tensor(out=aggs[:], in0=ps_w[:],
                                        in1=invb_sb[:, w0:w0 + 128],
                                        op=mybir.AluOpType.mult)
                W_sb = W1_sb if layer == 0 else W2_sb
                bias = b1_v if layer == 0 else b2_v
                psz = psa.tile([128, HID], f32, space="PSUM", tag="a")
                nc.tensor.matmul(out=psz[:], lhsT=aggs[:], rhs=W_sb[:],
                                 start=True, stop=False)
                nc.tensor.matmul(out=psz[:], lhsT=ones1b_v[0:1, :],
                                 rhs=bias[0:1, :], start=False, stop=True)
                if layer == 0:
                    # h1s = inv*relu(z): write straight into the layer-2
                    # self-row SBUF tile, then DMA that slice to the table
                    t2 = srow1[:][:, w, :]
                    nc.scalar.activation(t2, psz[:],
                                         mybir.ActivationFunctionType.Relu,
                                         scale=invs_sb[:, w:w + 1])
                    nc.sync.dma_start(tbl2_own[w0:w0 + 128, :], t2)
                else:
                    # h2 window [node, ch] -> pooled one-hot accumulate
                    h2w = ep.tile([128, HID], bf, tag="h2w")
                    nc.scalar.activation(h2w[:], psz[:],
                                         mybir.ActivationFunctionType.Relu)
                    sg = sp.tile([128, G], bf, tag="sg")
                    nc.vector.tensor_tensor(
                        out=sg[:],
                        in0=batch_sb[:, w:w + 1].to_broadcast([128, G]),
                        in1=iotag_sb[:],
                        op=mybir.AluOpType.is_equal)
                    nc.tensor.matmul(out=ps_pool[:], lhsT=h2w[:], rhs=sg[:],
                                     start=(w == 0), stop=(w == NW - 1),
                                     skip_group_check=True)

            agg_layer(0)
            agg_layer(1)

            # ---- pooled mean + head (replicated on every core)
            pooled_l = ep.tile([128, G], f32, tag="pool")
            nc.vector.tensor_copy(out=pooled_l[:], in_=ps_pool[:])
            ar_in = dp.tile([128, G], f32, tag="arin")
            ar_out = dp.tile([128, G], f32, addr_space="Shared", tag="arout")
            nc.sync.dma_start(ar_in[:], pooled_l[:])
            if no_coll:
                nc.sync.dma_start(ar_out[:], ar_in[:])
            else:
                nc.gpsimd.collective_compute(
                    "AllReduce", mybir.AluOpType.add,
                    ins=[ar_in.opt()], outs=[ar_out.opt()],
                    replica_groups=replica)
            pooled = ep.tile([128, G], f32, tag="pool")
            nc.sync.dma_start(pooled[:], ar_out[:])

            psc = psg.tile([128, G], f32, space="PSUM", tag="g")
            nc.tensor.matmul(out=psc[:], lhsT=ones1_v[0:1, :],
                             rhs=cnt_v[0:1, :], start=True, stop=True)
            cntb = ep.tile([128, G], f32, tag="pool")
            nc.vector.tensor_scalar_max(out=cntb[:], in0=psc[:], scalar1=1.0)
            invc = ep.tile([128, G], f32, tag="pool")
            nc.vector.reciprocal(invc[:], cntb[:])
            pmean = ep.tile([128, G], f32, tag="pool")
            nc.vector.tensor_tensor(out=pmean[:], in0=pooled[:], in1=invc[:],
                                    op=mybir.AluOpType.mult)

            psh1 = psg.tile([128, G], f32, space="PSUM", tag="g")
            nc.tensor.matmul(out=psh1[:], lhsT=Wh1_sb[:], rhs=pmean[:],
                             start=True, stop=True)
            relu1 = ep.tile([128, G], f32, tag="pool")
            nc.scalar.activation(relu1[:], psh1[:],
                                 mybir.ActivationFunctionType.Relu,
                                 bias=bh1_v[:, 0:1])
            psh2 = psg.tile([cfg.OUT, G], f32, space="PSUM", tag="g")
            nc.tensor.matmul(out=psh2[:], lhsT=Wh2_sb[:], rhs=relu1[:],
                             start=True, stop=True)
            out_sb = ep.tile([cfg.OUT, G], f32, tag="out")
            nc.vector.tensor_scalar_add(out=out_sb[:], in0=psh2[:],
                                        scalar1=bh2_v[0:cfg.OUT, 0:1])
            nc.sync.dma_start(t_out[:], out_sb[:])

    nc.compile()
    return nc


# ----------------------------------------------------------------- entry
def _run(inputs, cfg=CFG, trace=False):
    in_maps, meta = _host_prep(cfg, inputs["x"], inputs["edge_index"],
                               inputs["batch"])
    _add_weights(cfg, in_maps,
                 inputs["W1"], inputs["b1"], inputs["W2"], inputs["b2"],
                 inputs["Wh1"], inputs["bh1"], inputs["Wh2"], inputs["bh2"])
    nc = _build(cfg, meta)
    res = run_bass_kernel_spmd(nc, in_maps, list(range(cfg.C)), trace=trace)
    out = np.ascontiguousarray(np.asarray(res.results[0]["out"]).T)
    return out, res


def kernel(**inputs) -> np.ndarray:
    out, _ = _run(inputs, CFG, trace=False)
    return out


# revision 26
# speedup vs baseline: 1.6526x; 1.0175x over previous
"""Trainium2 Bass kernel for DemoGraphNet (2-layer GCN + mean-pool + MLP head).

Self-contained: hardcodes problem shapes and the 8-core sharding strategy.

v7 = v5 aggregation pipeline + quarter-chunked overlapped AllGather +
single packed constant blob (2 input args total).

Late-weight: aggregation commutes with the layer weight matmul
(A_hat @ (X W) = (A_hat @ X) W), so each layer gathers RAW feature rows
(layer 1: xs = D^-1/2 x from a replicated bf16 input table; layer 2: h1s
rows AllGathered per quarter) and applies the 128x128 weight + bias +
relu per 128-node dst window after aggregation in [dst, ch] orientation.

Quarter-chunked AllGather: the gather-table row space is QUARTER-major
(4 supertile-aligned window groups, each <= 32768 rows so int16 gather
indices still reach them).  Each quarter AllGathers as soon as layer 1
finishes its windows, overlapping the collective with the layer-1 tail;
layer-2 gather blocks == quarters, so only the last quarter's collective
can stall the layer-2 pipeline (~1/4 of the old exposure).

Packed blob: per-exec dispatch cost on this runtime scales with the
argument count (~70us/arg), dwarfing most device-side effects, so every
constant (gather indices, dst codes, one-hot iota bands, inv-sqrt
tables, weights, the layer-1 self-row image) is packed host-side into
one [128, B] uint8 image, DMA'd once, and sliced/bitcast on SBUF.

Edges live on their dst core (self-loops via identity matmul).
dma_gather (SWDGE) fetches 256B rows (bucket-sorted ascending for HBM
row locality) into [edge x chan] SBUF tiles.  Gathered rows carry
inv_sqrt[src]; window finish multiplies inv_sqrt[dst].  Pooled
per-graph sums are AllReduced; the tiny MLP head runs replicated.
"""

import math
import os
import sys

sys.path.insert(0, "/opt/trn_rl_repo")

import numpy as np
import ml_dtypes

import concourse.bass as bass
import concourse.mybir as mybir
import concourse.tile as tile
import concourse.bacc as bacc
from concourse import library_config
from concourse.bass_utils import run_bass_kernel_spmd

BF16 = ml_dtypes.bfloat16
F32 = np.float32


class Cfg:
    def __init__(self, n_nodes, n_graphs, st_w, n_cores=8, hid=128, out_c=8):
        assert n_nodes % n_cores == 0
        self.N = n_nodes
        self.G = n_graphs
        self.C = n_cores
        self.HID = hid
        self.OUT = out_c
        self.NPC = n_nodes // n_cores          # nodes per core
        self.WIN = 128                          # dst window width
        self.NW = math.ceil(self.NPC / 128)     # windows per core
        self.PAD_NPC = self.NW * 128
        self.ST_W = st_w                        # windows per supertile
        self.NST = math.ceil(self.NW / st_w)
        self.TBL_N = n_cores * self.PAD_NPC     # padded table rows
        self.NBLK = 4
        # split the NST supertiles into 4 quarters (last the smallest);
        # table layout is quarter-major so each quarter AllGathers
        # independently and is a contiguous int16-indexable gather block
        base = self.NST // 4
        rem = self.NST - 4 * base
        self.QSTS = [base + (1 if i < rem else 0) for i in range(4)]
        self.QW0 = [0]
        for i in range(4):
            self.QW0.append(min(self.QW0[-1] + self.QSTS[i] * st_w, self.NW))
        self.QWIN = [self.QW0[i + 1] - self.QW0[i] for i in range(4)]
        self.QROWS = [q * 128 * n_cores for q in self.QWIN]
        self.QROW0 = [0]
        for i in range(4):
            self.QROW0.append(self.QROW0[-1] + self.QROWS[i])
        assert all(r <= 32768 for r in self.QROWS), self.QROWS
        self.WQ_OF_W = []
        for i in range(4):
            self.WQ_OF_W += [i] * self.QWIN[i]


CFG = Cfg(n_nodes=100000, n_graphs=256,
          st_w=int(os.environ.get("GNN_STW", "4")))


# ------------------------------------------------------- blob layout
def _blob_layout(cfg, EP, NCH):
    """Column-byte layout of the packed constant image [128, total]."""
    sec = [
        ("idx",     np.int16, EP // 16),
        ("identb",  BF16,     128),
        ("dstcb",   BF16,     NCH),
        ("invb",    BF16,     cfg.PAD_NPC),
        ("invpp",   F32,      cfg.NW),
        ("batchpp", F32,      cfg.NW),
        ("iota256", BF16,     1280),
        ("iotag",   BF16,     cfg.G),
        ("cnt",     F32,      cfg.G),
        ("ones1",   F32,      128),
        ("ones1b",  BF16,     128),
        ("W1",      BF16,     cfg.HID),
        ("b1",      BF16,     cfg.HID),
        ("W2",      BF16,     cfg.HID),
        ("b2",      BF16,     cfg.HID),
        ("Wh1",     F32,      cfg.HID),
        ("bh1",     F32,      1),
        ("Wh2",     F32,      cfg.OUT),
        ("bh2",     F32,      1),
        ("xsown",   BF16,     cfg.NW * cfg.HID),
    ]
    layout = {}
    off = 0
    for name, dt, cols in sec:
        nbytes = cols * np.dtype(dt).itemsize
        layout[name] = (off, dt, cols)
        pad = 256 if name == "idx" else 4
        off += (nbytes + pad - 1) // pad * pad
    return layout, off


def _pack_blob(cfg, EP, NCH, pieces):
    layout, total = _blob_layout(cfg, EP, NCH)
    blob = np.zeros((128, total), dtype=np.uint8)
    for name, (off, dt, cols) in layout.items():
        arr = np.ascontiguousarray(pieces[name].astype(dt))
        assert arr.ndim == 2 and arr.shape[1] == cols, (name, arr.shape, cols)
        p = arr.shape[0]
        blob[0:p, off:off + cols * np.dtype(dt).itemsize] = arr.view(np.uint8)
    return blob


# ----------------------------------------------------------------- host prep
def _host_prep(cfg, x, edge_index, batch):
    """Shard + marshal inputs. Index bookkeeping plus input re-encoding
    (node->slot placement, scaling x rows by inv_sqrt and casting to bf16);
    every matmul and all aggregation FLOPs run on device."""
    N, C = cfg.N, cfg.C
    NPC, WIN, NW, ST_W, NST = cfg.NPC, cfg.WIN, cfg.NW, cfg.ST_W, cfg.NST
    NBLK, PAD_NPC = cfg.NBLK, cfg.PAD_NPC
    TBL_N = cfg.TBL_N

    src = np.asarray(edge_index[0], dtype=np.int64)
    dst = np.asarray(edge_index[1], dtype=np.int64)
    batch = np.asarray(batch, dtype=np.int64)
    x = np.asarray(x, dtype=np.float32)

    deg = (np.bincount(dst, minlength=N) + 1).astype(np.float64)
    inv_sqrt = (1.0 / np.sqrt(deg)).astype(np.float32)

    # ---- balanced node -> slot placement.  Pass 1: degree-sorted snake
    # over all (core, window) slots equalizes window totals.  Pass 2:
    # within each src-block's quarter, re-deal nodes to windows matching
    # per-block in-degree vectors to the per-round target, so per-(window,
    # src-block) bucket counts are near-uniform across cores too.
    NWT = TBL_N // WIN                      # total windows (all cores)
    order = np.argsort(-deg, kind="stable")
    slot_of_node = np.empty(N, dtype=np.int64)
    nfull = N // NWT
    for r in range(nfull):
        nodes_r = order[r * NWT:(r + 1) * NWT]
        wins = np.arange(NWT) if r % 2 == 0 else np.arange(NWT)[::-1]
        slot_of_node[nodes_r] = wins * WIN + r
    rem = N - nfull * NWT
    if rem:
        nodes_r = order[nfull * NWT:]
        wins = np.arange(rem) if nfull % 2 == 0 else NWT - 1 - np.arange(rem)
        slot_of_node[nodes_r] = wins * WIN + nfull

    # per-node per-src-block in-degree under pass-1 source placement
    # (pass 2 keeps every node inside its quarter, so src blocks are stable)
    wq_of_w = np.asarray(cfg.WQ_OF_W, dtype=np.int64)   # local window -> blk
    src_blk1 = wq_of_w[(slot_of_node[src] // WIN) % NW]
    dvec = np.zeros((N, NBLK), dtype=np.int64)
    np.add.at(dvec, (dst, src_blk1), 1)

    gwin_all = np.arange(TBL_N // WIN)
    for q in range(NBLK):
        qwins = gwin_all[wq_of_w[gwin_all % NW] == q]
        wpq = len(qwins)
        qnodes = np.nonzero(
            wq_of_w[(slot_of_node // WIN) % NW] == q)[0]
        qn = qnodes[np.argsort(-deg[qnodes], kind="stable")]
        dv = dvec[qn].astype(np.float64)            # [nq, NBLK]
        nq = len(qn)
        S = np.zeros((wpq, NBLK))
        fill = np.zeros(wpq, dtype=np.int64)
        new_slot = np.empty(nq, dtype=np.int64)
        pos = 0
        rnd = 0
        while pos < nq:
            take = min(wpq, nq - pos)
            rn = np.arange(pos, pos + take)
            avail = np.ones(wpq, dtype=bool)
            avail[fill >= WIN] = False
            # deficit-matching: most-imbalanced nodes first, window with
            # the largest deficit in the node's dominant block
            node_order = rn[np.argsort(-np.abs(dv[rn] - dv[rn].mean(0)).max(1))]
            target = S[avail].mean(0) if avail.any() else S.mean(0)
            D = (target[None, :] - S)                # deficit per window
            for i in node_order:
                b = int(np.argmax(dv[i] - dv[rn].mean(0)))
                cand = np.nonzero(avail)[0]
                wsel = cand[np.argmax(D[cand, b])]
                new_slot[i] = qwins[wsel] * WIN + fill[wsel]
                S[wsel] += dv[i]
                D[wsel] -= dv[i]
                fill[wsel] += 1
                avail[wsel] = fill[wsel] < WIN
            pos += take
            rnd += 1
        slot_of_node[qn] = new_slot

    node_of_slot = np.full(TBL_N, -1, dtype=np.int64)
    node_of_slot[slot_of_node] = np.arange(N)
    valid_slot = node_of_slot >= 0

    # quarter-major table row for every (old-numbering) global slot
    g_of_s = np.arange(TBL_N) // WIN
    i_of_s = np.arange(TBL_N) % WIN
    c_of_s = g_of_s // NW
    w_of_s = g_of_s % NW
    q_of_s = wq_of_w[w_of_s]
    qw0 = np.asarray(cfg.QW0[:4], dtype=np.int64)
    qwin = np.asarray(cfg.QWIN, dtype=np.int64)
    qrow0 = np.asarray(cfg.QROW0[:4], dtype=np.int64)
    tablerow_of_slot = (qrow0[q_of_s] + c_of_s * qwin[q_of_s] * WIN
                        + (w_of_s - qw0[q_of_s]) * WIN + i_of_s)

    # self-loops (the +I in A_hat) are NOT streamed as edges: each window
    # finish adds its own rows via an identity matmul instead (keeps the
    # per-(window, src-block) bucket counts balanced across cores).
    src_slot = slot_of_node[src]
    dst_slotg = slot_of_node[dst]
    core = dst_slotg // PAD_NPC
    tblrow = tablerow_of_slot[src_slot]
    blk = wq_of_w[(src_slot // WIN) % NW]
    w_loc = (dst_slotg % PAD_NPC) // WIN
    slot_in_win = dst_slotg % WIN

    # bucket quotas: max count over cores, rounded to 16
    key = (core * NW + w_loc) * NBLK + blk
    counts = np.bincount(key, minlength=C * NW * NBLK).reshape(C, NW, NBLK)
    quota = counts.max(axis=0)
    quota = ((quota + 15) // 16) * 16          # [NW, NBLK]

    # segment layout: stream ordered (st, blk, w); segments pad to 128
    bucket_base = np.zeros((NW, NBLK), dtype=np.int64)
    seg_off = np.zeros((NST, NBLK), dtype=np.int64)
    seg_len = np.zeros((NST, NBLK), dtype=np.int64)
    st_meta = []                                # per st: emission metadata
    pos = 0
    for st in range(NST):
        ws = list(range(st * ST_W, min((st + 1) * ST_W, NW)))
        seg_groups = {}
        for b in range(NBLK):
            seg_off[st, b] = pos
            bounds = []
            p0 = pos
            for w in ws:
                bucket_base[w, b] = pos
                bounds.append((w, pos - p0, pos - p0 + quota[w, b]))
                pos += quota[w, b]
            L = (pos - p0 + 127) // 128 * 128
            pos = p0 + L
            seg_len[st, b] = L
            gci0 = p0 // 128
            # classify chunks: interior (one window) vs boundary (two)
            groups = []          # (mode, gci, gl, [(k_or_band, w)...])
            k = 0
            nch_seg = L // 128
            while k < nch_seg:
                lo, hi = 128 * k, 128 * (k + 1)
                ov = [w for (w, s, e) in bounds if s < hi and lo < e]
                assert 1 <= len(ov) <= 2, (st, b, k, ov)
                if len(ov) == 2:
                    weven = ov[0] if ov[0] % 2 == 0 else ov[1]
                    wodd = ov[1] if ov[1] % 2 == 1 else ov[0]
                    assert weven % 2 == 0 and wodd % 2 == 1
                    groups.append(("bnd", gci0 + k, 1,
                                   [(0, weven), (1, wodd)]))
                    k += 1
                else:
                    w = ov[0]
                    j = k
                    while j < nch_seg and j - k < 4:
                        lo2, hi2 = 128 * j, 128 * (j + 1)
                        ov2 = [ww for (ww, s, e) in bounds
                               if s < hi2 and lo2 < e]
                        if ov2 != [w]:
                            break
                        j += 1
                    groups.append(("int", gci0 + k, j - k,
                                   [(kk, w) for kk in range(j - k)]))
                    k = j
            seg_groups[b] = groups
        # first/last matmul flags per window within this st
        totals = {}
        for b in range(NBLK):
            for (_, _, _, mms) in seg_groups[b]:
                for (_, w) in mms:
                    totals[w] = totals.get(w, 0) + 1
        st_meta.append(dict(groups=seg_groups, totals=totals))
    EP = pos
    NCH = EP // 128

    cnt = np.bincount(batch, minlength=cfg.G).astype(np.float32)

    # replicated layer-1 gather table: xs = inv_sqrt * x, quarter-major rows
    xs = (x * inv_sqrt[:, None]).astype(BF16)
    xs_slot = np.zeros((TBL_N, cfg.HID), dtype=BF16)
    xs_slot[valid_slot] = xs[node_of_slot[valid_slot]]
    xs_pad = np.zeros((TBL_N, cfg.HID), dtype=BF16)
    xs_pad[tablerow_of_slot] = xs_slot

    inv_slot = np.ones(TBL_N, dtype=np.float32)
    inv_slot[valid_slot] = inv_sqrt[node_of_slot[valid_slot]]
    batch_slot = np.full(TBL_N, -1.0, dtype=np.float32)
    batch_slot[valid_slot] = batch[node_of_slot[valid_slot]].astype(np.float32)

    in_maps = []
    okey = ((w_loc // ST_W) * NBLK + blk) * NW + w_loc
    for c in range(C):
        sel = np.nonzero(core == c)[0]
        e_okey = okey[sel]
        e_row = tblrow[sel]
        order_e = np.lexsort((e_row, e_okey))
        sel = sel[order_e]
        # rank within bucket
        bkey = (w_loc[sel] * NBLK + blk[sel])
        change = np.ones(len(sel), dtype=bool)
        change[1:] = bkey[1:] != bkey[:-1]
        gstart = np.maximum.accumulate(np.where(change, np.arange(len(sel)), 0))
        rank = np.arange(len(sel)) - gstart
        posn = bucket_base[w_loc[sel], blk[sel]] + rank

        e_idx16 = np.zeros(EP, dtype=np.int16)
        e_dst = np.full(EP, -1.0, dtype=np.float32)
        e_idx16[posn] = (tblrow[sel] - qrow0[blk[sel]]).astype(np.int16)
        e_dst[posn] = (slot_in_win[sel]
                       + WIN * (w_loc[sel] % 2)).astype(np.float32)

        idx_img = np.tile(e_idx16.reshape(-1, 16).T, (8, 1)).copy()
        dst_colb = np.ascontiguousarray(e_dst.reshape(NCH, 128).T).astype(BF16)

        lo = c * PAD_NPC
        inv_own = inv_slot[lo:lo + PAD_NPC]
        inv_pp = inv_own.reshape(NW, 128).T.copy()
        inv_b = np.broadcast_to(inv_own, (128, PAD_NPC)).astype(BF16).copy()
        batch_pp = batch_slot[lo:lo + PAD_NPC].reshape(NW, 128).T.copy()
        # layer-1 self rows, pre-arranged [128 slot-in-window, NW*HID]
        xsown_img = (xs_slot[lo:lo + PAD_NPC]
                     .reshape(NW, 128, cfg.HID).transpose(1, 0, 2)
                     .reshape(128, NW * cfg.HID).copy())

        in_maps.append({
            "_xs": xs_pad,
            "_pieces": {
                "idx": idx_img, "identb": np.eye(128, dtype=np.float32),
                "dstcb": dst_colb, "invb": inv_b, "invpp": inv_pp,
                "batchpp": batch_pp,
                "iota256": np.broadcast_to(
                    np.tile(np.arange(256, dtype=np.float32), 5),
                    (128, 1280)),
                "iotag": np.broadcast_to(
                    np.arange(cfg.G, dtype=np.float32), (128, cfg.G)),
                "cnt": cnt[None, :],
                "ones1": np.ones((1, 128), dtype=np.float32),
                "ones1b": np.ones((1, 128), dtype=np.float32),
                "xsown": xsown_img,
            },
        })

    meta = dict(EP=EP, NCH=NCH, seg_off=seg_off, seg_len=seg_len,
                st_meta=st_meta)
    return in_maps, meta


def _add_weights(cfg, in_maps, W1, b1, W2, b2, Wh1, bh1, Wh2, bh2):
    wts = {
        "W1": np.asarray(W1, F32), "b1": np.asarray(b1, F32).reshape(1, -1),
        "W2": np.asarray(W2, F32), "b2": np.asarray(b2, F32).reshape(1, -1),
        "Wh1": np.asarray(Wh1, F32),
        "bh1": np.asarray(bh1, F32).reshape(-1, 1),
        "Wh2": np.asarray(Wh2, F32),
        "bh2": np.asarray(bh2, F32).reshape(-1, 1),
    }
    for m in in_maps:
        pieces = m.pop("_pieces")
        pieces.update(wts)
        NCH = pieces["dstcb"].shape[1]
        EP = NCH * 128
        blob = _pack_blob(cfg, EP, NCH, pieces)
        bpad = (-blob.shape[1]) % 256
        if bpad:
            blob = np.concatenate(
                [blob, np.zeros((128, bpad), np.uint8)], axis=1)
        # blob bytes as extra bf16 table rows: partition-major chunks
        brows = blob.view(BF16).reshape(128 * (blob.shape[1] // 256), 128)
        xs_pad = m.pop("_xs")
        m["mega"] = np.concatenate([xs_pad, brows], axis=0)


# ------------------------------------------------------------- program build
def _build(cfg, meta):
    NW, ST_W, NST, NBLK = cfg.NW, cfg.ST_W, cfg.NST, cfg.NBLK
    NCH, EP = meta["NCH"], meta["EP"]
    HID, G = cfg.HID, cfg.G
    bf = mybir.dt.bfloat16
    f32 = mybir.dt.float32
    u8 = mybir.dt.uint8

    no_coll = os.environ.get("GNN_NO_COLL") == "1"
    no_gather = os.environ.get("GNN_NO_GATHER") == "1"
    nc = bacc.Bacc("TRN2", target_bir_lowering=False, debug=False,
                   num_devices=cfg.C, num_swdge_queues=4)

    layout, blob_bytes = _blob_layout(cfg, EP, NCH)
    blob_bytes += (-blob_bytes) % 256
    blob_rows = 128 * (blob_bytes // 256)
    P_mega = nc.declare_dram_parameter(
        "mega", [cfg.TBL_N + blob_rows, HID], bf, isOutput=False)
    t_out = nc.declare_dram_parameter("out", [cfg.OUT, G], f32, isOutput=True)

    replica = [list(range(cfg.C))]
    mdt = {np.int16: mybir.dt.int16, BF16: bf, F32: f32}

    with tile.TileContext(nc) as tc:
        with (
            tc.tile_pool(name="const", bufs=1) as cp,
            tc.tile_pool(name="mtiles",
                         bufs=int(os.environ.get("GNN_MPBUFS", "8"))) as mp,
            tc.tile_pool(name="stiles", bufs=12) as sp,
            tc.tile_pool(name="evac", bufs=6) as ep,
            tc.tile_pool(name="psw",
                         bufs=int(os.environ.get("GNN_PSWBUFS", "4")),
                         space="PSUM") as psw,
            tc.tile_pool(name="psa",
                         bufs=int(os.environ.get("GNN_PSABUFS", "3")),
                         space="PSUM") as psa,
            tc.tile_pool(name="psg", bufs=1, space="PSUM") as psg,
            tc.tile_pool(name="dram", bufs=1, space="DRAM") as dp,
        ):
            nc.gpsimd.load_library(library_config.mlp)

            # ---- the packed constants: idx section DMA'd first so the
            # layer-1 gathers start without waiting for the full blob
            blob_sb = cp.tile([128, blob_bytes], u8, tag="c_blob",
                              name="c_blob")
            mega_u8 = (P_mega[:].bitcast(u8)
                       [cfg.TBL_N:cfg.TBL_N + blob_rows, :]
                       .rearrange("(p r) c -> p (r c)", p=128))
            idx_end = ((layout["idx"][2] * 2) + 255) // 256 * 256
            if os.environ.get("GNN_NO_BLOBSPLIT") == "1":
                nc.sync.dma_start(blob_sb[:], mega_u8)
            else:
                nc.sync.dma_start(blob_sb[:, 0:idx_end],
                                  mega_u8[:, 0:idx_end])
                nc.sync.dma_start(blob_sb[:, idx_end:blob_bytes],
                                  mega_u8[:, idx_end:blob_bytes])

            def view(name):
                off, dt, cols = layout[name]
                nb = cols * np.dtype(dt).itemsize
                return blob_sb[:, off:off + nb].bitcast(mdt[dt])

            idx_sb = view("idx")
            ident_sb = view("identb")
            dstb_sb = view("dstcb")
            invb_sb = view("invb")
            invs_sb = view("invpp")
            batch_sb = view("batchpp")
            iota256_sb = view("iota256")
            iotag_sb = view("iotag")
            cnt_v = view("cnt")
            ones1_v = view("ones1")
            ones1b_v = view("ones1b")
            W1_sb = view("W1"); b1_v = view("b1")
            W2_sb = view("W2"); b2_v = view("b2")
            Wh1_sb = view("Wh1"); bh1_v = view("bh1")
            Wh2_sb = view("Wh2"); bh2_v = view("bh2")
            srow0 = view("xsown").rearrange("p (w c) -> p w c", c=HID)

            # layer-2 gather table (h1s rows), own shard + AllGathered in
            # four quarter-major chunks so the collective overlaps layer-1
            # tail compute.  Raw dram tensors, manually registered in the
            # DGE table (SWDGE descriptor relocation needs it).
            tbl2_own = nc.dram_tensor("tbl2own", [cfg.PAD_NPC, HID], bf)
            tbl2q = [nc.dram_tensor(f"tbl2q{q}", [cfg.QROWS[q], HID], bf)
                     for q in range(4)]
            for t in (P_mega, *tbl2q):
                mloc = nc.lookup_mloc(t)
                if mloc.table_entry_id is None:
                    mloc.table_entry_id = len(nc.dge_table) + 1
                    nc.dge_table.append(mloc.name)

            def allgather_quarter(q):
                lo0 = cfg.QW0[q] * 128
                hi0 = (cfg.QW0[q] + cfg.QWIN[q]) * 128
                if no_coll:
                    nc.sync.dma_start(tbl2q[q][0:hi0 - lo0, :],
                                      tbl2_own[lo0:hi0, :])
                else:
                    nc.gpsimd.collective_compute(
                        "AllGather", mybir.AluOpType.bypass,
                        ins=[tbl2_own[lo0:hi0, :]], outs=[tbl2q[q][:]],
                        replica_groups=replica)

            ps_pool = psg.tile([128, G], f32, space="PSUM", tag="g")

            # layer-2 self rows: written in place by layer-1 window finish
            srow1 = cp.tile([128, NW, 128], bf, tag="c_srow1", name="c_srow1")

            wslots = {}

            def mm(layer, w, lhsT, rhs, remaining, full):
                first = remaining[w] == full[w]
                last = remaining[w] == 1
                remaining[w] -= 1
                nc.tensor.matmul(out=wslots[w][:], lhsT=lhsT, rhs=rhs,
                                 start=first, stop=False,
                                 skip_group_check=True)
                if last:
                    finish_window(layer, w, wslots[w][:])

            # ---- aggregation sweep (shared for both layers)
            def agg_layer(layer):
                q_end_sts = []
                acc = 0
                for q in range(4):
                    acc += cfg.QSTS[q]
                    q_end_sts.append(acc - 1)
                for st in range(NST):
                    sm = meta["st_meta"][st]
                    remaining = dict(sm["totals"])
                    full = sm["totals"]
                    wslots.clear()
                    wslots.update({w: psw.tile([128, 128], f32, space="PSUM",
                                               tag="win",
                                               name=f"wt_{layer}_{w}")
                                   for w in range(st * ST_W,
                                                  min((st + 1) * ST_W, NW))})
                    m_tiles = {}
                    for b in range(NBLK):
                        off = int(meta["seg_off"][st, b])
                        ln = int(meta["seg_len"][st, b])
                        if ln == 0:
                            continue
                        mt = mp.tile([128, ln // 128, 128], bf, tag="mtile",
                                     name=f"m_{layer}_{st}_{b}")
                        m_tiles[b] = (mt, off // 128)
                        if no_gather:
                            nc.gpsimd.memset(mt[:], 0.5)
                        else:
                            tsl = (P_mega[cfg.QROW0[b]:
                                          cfg.QROW0[b + 1], :]
                                   if layer == 0 else tbl2q[b][:])
                            nc.gpsimd.dma_gather(
                                mt[:], tsl,
                                idx_sb[:, off // 16: off // 16 + ln // 16],
                                ln, ln, HID,
                                single_packet=False, queue_num=b % 4)
                    for b in range(NBLK):
                        if b not in m_tiles:
                            continue
                        mt, mbase = m_tiles[b]
                        for (mode, gci0, gl, mms) in sm["groups"][b]:
                            if mode == "int":
                                w = mms[0][1]
                                p = w % 2
                                sq = sp.tile([128, max(gl, 1), 128], bf,
                                             tag="s",
                                             name=f"s_{layer}_{st}_{gci0}")
                                nc.vector.tensor_tensor(
                                    out=sq[:],
                                    in0=dstb_sb[:, gci0:gci0 + gl].to_broadcast(
                                        [128, gl, 128]),
                                    in1=iota256_sb[:, p * 128:
                                                   p * 128 + gl * 256].rearrange(
                                        "p (c j) -> p c j", j=256)[:, :, 0:128],
                                    op=mybir.AluOpType.is_equal)
                                for k in range(gl):
                                    mm(layer, w, mt[:, gci0 + k - mbase, :],
                                       sq[:, k, :], remaining, full)
                            else:
                                gci = gci0
                                sq = sp.tile([128, 2, 128], bf, tag="s",
                                             name=f"s_{layer}_{st}_{gci}_b")
                                nc.vector.tensor_tensor(
                                    out=sq[:],
                                    in0=dstb_sb[:, gci:gci + 1].to_broadcast(
                                        [128, 2, 128]),
                                    in1=iota256_sb[:, 0:256].rearrange(
                                        "p (c j) -> p c j", j=128),
                                    op=mybir.AluOpType.is_equal)
                                for (k, w) in mms:
                                    mm(layer, w, mt[:, gci - mbase, :],
                                       sq[:, k, :], remaining, full)
                    if layer == 0 and st in q_end_sts:
                        allgather_quarter(q_end_sts.index(st))

            def finish_window(layer, w, ps_w):
                # ps_w: [ch_in, dst] aggregate of gathered rows (carrying
                # inv_sqrt[src]); add the self-loop rows (own window, via
                # identity matmul transpose-accumulate), then evac applies
                # inv_sqrt[dst]; the layer weight + bias + relu run per
                # window in [dst, ch] layout.
                w0 = w * 128
                nc.tensor.matmul(out=ps_w,
                                 lhsT=(srow0 if layer == 0
                                       else srow1[:])[:, w, :],
                                 rhs=ident_sb[:],
                                 start=False, stop=True,
                                 skip_group_check=True)
                aggs = ep.tile([128, 128], bf, tag="aggs")
                nc.vector.tensor_tensor(out=aggs[:], in0=ps_w[:],
                                        in1=invb_sb[:, w0:w0 + 128],
                                        op=mybir.AluOpType.mult)
                W_sb = W1_sb if layer == 0 else W2_sb
                bias = b1_v if layer == 0 else b2_v
                psz = psa.tile([128, HID], f32, space="PSUM", tag="a")
                nc.tensor.matmul(out=psz[:], lhsT=aggs[:], rhs=W_sb[:],
                                 start=True, stop=False)
                nc.tensor.matmul(out=psz[:], lhsT=ones1b_v[0:1, :],
                                 rhs=bias[0:1, :], start=False, stop=True)
                if layer == 0:
                    # h1s = inv*relu(z): write straight into the layer-2
                    # self-row SBUF tile, then DMA that slice to the table
                    t2 = srow1[:][:, w, :]
                    nc.scalar.activation(t2, psz[:],
                                         mybir.ActivationFunctionType.Relu,
                                         scale=invs_sb[:, w:w + 1])
                    nc.sync.dma_start(tbl2_own[w0:w0 + 128, :], t2)
                else:
                    # h2 window [node, ch] -> pooled one-hot accumulate
                    h2w = ep.tile([128, HID], bf, tag="h2w")
                    nc.scalar.activation(h2w[:], psz[:],
                                         mybir.ActivationFunctionType.Relu)
                    sg = sp.tile([128, G], bf, tag="sg")
                    nc.vector.tensor_tensor(
                        out=sg[:],
                        in0=batch_sb[:, w:w + 1].to_broadcast([128, G]),
                        in1=iotag_sb[:],
                        op=mybir.AluOpType.is_equal)
                    nc.tensor.matmul(out=ps_pool[:], lhsT=h2w[:], rhs=sg[:],
                                     start=(w == 0), stop=(w == NW - 1),
                                     skip_group_check=True)

            agg_layer(0)
            agg_layer(1)

            # ---- pooled mean + head (replicated on every core)
            pooled_l = ep.tile([128, G], f32, tag="pool")
            nc.vector.tensor_copy(out=pooled_l[:], in_=ps_pool[:])
            ar_in = dp.tile([128, G], f32, tag="arin")
            ar_out = dp.tile([128, G], f32, addr_space="Shared", tag="arout")
            nc.sync.dma_start(ar_in[:], pooled_l[:])
            if no_coll:
                nc.sync.dma_start(ar_out[:], ar_in[:])
            else:
                nc.gpsimd.collective_compute(
                    "AllReduce", mybir.AluOpType.add,
                    ins=[ar_in.opt()], outs=[ar_out.opt()],
                    replica_groups=replica)
            pooled = ep.tile([128, G], f32, tag="pool")
            nc.sync.dma_start(pooled[:], ar_out[:])

            psc = psg.tile([128, G], f32, space="PSUM", tag="g")
            nc.tensor.matmul(out=psc[:], lhsT=ones1_v[0:1, :],
                             rhs=cnt_v[0:1, :], start=True, stop=True)
            cntb = ep.tile([128, G], f32, tag="pool")
            nc.vector.tensor_scalar_max(out=cntb[:], in0=psc[:], scalar1=1.0)
            invc = ep.tile([128, G], f32, tag="pool")
            nc.vector.reciprocal(invc[:], cntb[:])
            pmean = ep.tile([128, G], f32, tag="pool")
            nc.vector.tensor_tensor(out=pmean[:], in0=pooled[:], in1=invc[:],
                                    op=mybir.AluOpType.mult)

            psh1 = psg.tile([128, G], f32, space="PSUM", tag="g")
            nc.tensor.matmul(out=psh1[:], lhsT=Wh1_sb[:], rhs=pmean[:],
                             start=True, stop=True)
            relu1 = ep.tile([128, G], f32, tag="pool")
            nc.scalar.activation(relu1[:], psh1[:],
                                 mybir.ActivationFunctionType.Relu,
                                 bias=bh1_v[:, 0:1])
            psh2 = psg.tile([cfg.OUT, G], f32, space="PSUM", tag="g")
            nc.tensor.matmul(out=psh2[:], lhsT=Wh2_sb[:], rhs=relu1[:],
                             start=True, stop=True)
            out_sb = ep.tile([cfg.OUT, G], f32, tag="out")
            nc.vector.tensor_scalar_add(out=out_sb[:], in0=psh2[:],
                                        scalar1=bh2_v[0:cfg.OUT, 0:1])
            nc.sync.dma_start(t_out[:], out_sb[:])

    nc.compile()
    return nc


# ----------------------------------------------------------------- entry
def _run(inputs, cfg=CFG, trace=False):
    in_maps, meta = _host_prep(cfg, inputs["x"], inputs["edge_index"],
                               inputs["batch"])
    _add_weights(cfg, in_maps,
                 inputs["W1"], inputs["b1"], inputs["W2"], inputs["b2"],
                 inputs["Wh1"], inputs["bh1"], inputs["Wh2"], inputs["bh2"])
    nc = _build(cfg, meta)
    res = run_bass_kernel_spmd(nc, in_maps, list(range(cfg.C)), trace=trace)
    out = np.ascontiguousarray(np.asarray(res.results[0]["out"]).T)
    return out, res


def kernel(**inputs) -> np.ndarray:
    out, _ = _run(inputs, CFG, trace=False)
    return out
